# revision 1
# baseline (speedup 1.0000x reference)
"""AttentionDecoder Trainium2 kernel (8 NeuronCores).

Strategy:
  - Batch-shard the sequential recurrence: core c owns batches [4c, 4c+4).
    Attention (scores/softmax/context) + 2-layer GRU run per-core on 4
    batches; activations are kept transposed ([d, b], partition=d) where
    a matmul needs that dim contracted, with PE-transposes for the small
    [4, 512] tensors that cross between the two layouts.
  - The vocab projection never feeds back into the recurrence (teacher
    forcing), so per-step outputs Y_u = [h1_u; ctx_u] are staged to DRAM
    (transposed), AllGathered once (1 MB/rank), and the [2048, 1024] x
    [1024, 4000] projection runs vocab-sharded at the end.
  - GRU weights live in SBUF as bf16 (halves footprint, same PE rate);
    attention + projection matmuls use float32r (1 cycle/row at N>=256).
  - r/z gates of both GRU layers use a fused [x; h] contraction.
"""

import numpy as np
import ml_dtypes

import concourse.bass as bass
import concourse.bacc as bacc_mod
import concourse.mybir as mybir
from concourse import tile
from concourse.bass_utils import run_bass_kernel_spmd

B, T, U = 32, 512, 64
V, H, E = 32000, 512, 512
NCORES = 8
BL = B // NCORES          # local batches per core
VS = V // NCORES          # vocab shard
VSP = 4096                # padded vocab shard (32 * 128)
UB = U * B                # 2048 output rows
UBL = U * BL              # 256 local Y columns

F32 = mybir.dt.float32
F32R = mybir.dt.float32r
BF16 = mybir.dt.bfloat16
AX = mybir.AxisListType
ALU = mybir.AluOpType
ACTF = mybir.ActivationFunctionType


def r32(ap):
    return ap.bitcast(F32R)


def build_nc(u_steps=U):
    nc = bacc_mod.Bacc()

    encE_d = nc.declare_dram_parameter("encE", [BL, E, T], F32R, isOutput=False)
    encT_d = nc.declare_dram_parameter("encT", [BL, T, E], F32R, isOutput=False)
    embT_d = nc.declare_dram_parameter("embT", [U, H, BL], F32, isOutput=False)
    mask_d = nc.declare_dram_parameter("mask", [128, T], F32, isOutput=False)
    wattn_d = nc.declare_dram_parameter("wattn", [H, E], F32, isOutput=False)
    wrz0_d = nc.declare_dram_parameter("wrz0", [12, 128, 2 * H], BF16, isOutput=False)
    wn0i_d = nc.declare_dram_parameter("wn0i", [8, 128, H], BF16, isOutput=False)
    wn0h_d = nc.declare_dram_parameter("wn0h", [4, 128, H], BF16, isOutput=False)
    wrz1_d = nc.declare_dram_parameter("wrz1", [8, 128, 2 * H], BF16, isOutput=False)
    wn1i_d = nc.declare_dram_parameter("wn1i", [4, 128, H], BF16, isOutput=False)
    wn1h_d = nc.declare_dram_parameter("wn1h", [4, 128, H], BF16, isOutput=False)
    brz0_d = nc.declare_dram_parameter("brz0", [BL, 2 * H], F32, isOutput=False)
    bn0i_d = nc.declare_dram_parameter("bn0i", [BL, H], F32, isOutput=False)
    bn0h_d = nc.declare_dram_parameter("bn0h", [BL, H], F32, isOutput=False)
    brz1_d = nc.declare_dram_parameter("brz1", [BL, 2 * H], F32, isOutput=False)
    bn1i_d = nc.declare_dram_parameter("bn1i", [BL, H], F32, isOutput=False)
    bn1h_d = nc.declare_dram_parameter("bn1h", [BL, H], F32, isOutput=False)
    woutT_d = nc.declare_dram_parameter("woutT", [8, 128, VSP], F32R, isOutput=False)
    bout_d = nc.declare_dram_parameter("bout", [128, VSP // 128], F32, isOutput=False)
    ident_d = nc.declare_dram_parameter("ident", [128, 128], F32, isOutput=False)
    out_d = nc.declare_dram_parameter("out", [VSP, UB], F32, isOutput=True)

    with tile.TileContext(nc) as tc:
        with (
            tc.tile_pool(name="res", bufs=1) as res,
            tc.tile_pool(name="dram", bufs=1, space="DRAM") as dram,
        ):
            bout_sb = res.tile([128, VSP // 128], F32, tag="bout")
            nc.sync.dma_start(bout_sb[:], bout_d.ap())

            ytl = dram.tile([8, 128, UBL], F32, tag="ytl")
            yts = dram.tile([64, 128, UBL], F32, tag="yts", addr_space="Shared")

            # ---- phase 1: recurrence ----
            with (
                tc.tile_pool(name="p1r", bufs=1) as p1r,
                tc.tile_pool(name="p1s", bufs=2) as p1s,
                tc.tile_pool(name="p1g", bufs=1) as p1g,
                tc.tile_pool(name="p1p", bufs=2, space="PSUM") as p1p,
                tc.tile_pool(name="p1q", bufs=1, space="PSUM") as p1q,
            ):
                # phase-1 resident SBUF tensors
                encE_sb = p1r.tile([128, 4, BL, T], F32R, tag="encE")
                for c4 in range(4):
                    nc.sync.dma_start(
                        encE_sb[:, c4, :, :],
                        encE_d.ap()[:, c4 * 128:(c4 + 1) * 128, :].transpose([1, 0, 2]))
                encT_sb = p1r.tile([128, 4, BL, E], F32R, tag="encT")
                for c4 in range(4):
                    nc.sync.dma_start(
                        encT_sb[:, c4, :, :],
                        encT_d.ap()[:, c4 * 128:(c4 + 1) * 128, :].transpose([1, 0, 2]))
                embT_sb = p1r.tile([128, 4, U, BL], F32, tag="embT")
                for c4 in range(4):
                    nc.sync.dma_start(
                        embT_sb[:, c4, :, :],
                        embT_d.ap()[:, c4 * 128:(c4 + 1) * 128, :].transpose([1, 0, 2]))
                mask_sb = p1r.tile([128, T], F32, tag="mask")
                nc.sync.dma_start(mask_sb[:], mask_d.ap())
                wattn_sb = p1r.tile([128, 4, E], F32, tag="wattn")
                for c4 in range(4):
                    nc.sync.dma_start(wattn_sb[:, c4, :],
                                      wattn_d.ap()[c4 * 128:(c4 + 1) * 128, :])
                wrz0_sb = p1r.tile([128, 12, 2 * H], BF16, tag="wrz0")
                nc.sync.dma_start(wrz0_sb[:], wrz0_d.ap().transpose([1, 0, 2]))
                wn0i_sb = p1r.tile([128, 8, H], BF16, tag="wn0i")
                nc.sync.dma_start(wn0i_sb[:], wn0i_d.ap().transpose([1, 0, 2]))
                wn0h_sb = p1r.tile([128, 4, H], BF16, tag="wn0h")
                nc.sync.dma_start(wn0h_sb[:], wn0h_d.ap().transpose([1, 0, 2]))
                wrz1_sb = p1r.tile([128, 8, 2 * H], BF16, tag="wrz1")
                nc.sync.dma_start(wrz1_sb[:], wrz1_d.ap().transpose([1, 0, 2]))
                wn1i_sb = p1r.tile([128, 4, H], BF16, tag="wn1i")
                nc.sync.dma_start(wn1i_sb[:], wn1i_d.ap().transpose([1, 0, 2]))
                wn1h_sb = p1r.tile([128, 4, H], BF16, tag="wn1h")
                nc.sync.dma_start(wn1h_sb[:], wn1h_d.ap().transpose([1, 0, 2]))
                brz0_sb = p1r.tile([BL, 2 * H], F32, tag="brz0")
                nc.sync.dma_start(brz0_sb[:], brz0_d.ap())
                bn0i_sb = p1r.tile([BL, H], F32, tag="bn0i")
                nc.sync.dma_start(bn0i_sb[:], bn0i_d.ap())
                bn0h_sb = p1r.tile([BL, H], F32, tag="bn0h")
                nc.sync.dma_start(bn0h_sb[:], bn0h_d.ap())
                brz1_sb = p1r.tile([BL, 2 * H], F32, tag="brz1")
                nc.sync.dma_start(brz1_sb[:], brz1_d.ap())
                bn1i_sb = p1r.tile([BL, H], F32, tag="bn1i")
                nc.sync.dma_start(bn1i_sb[:], bn1i_d.ap())
                bn1h_sb = p1r.tile([BL, H], F32, tag="bn1h")
                nc.sync.dma_start(bn1h_sb[:], bn1h_d.ap())
                ident_sb = p1r.tile([128, 128], F32, tag="ident")
                nc.sync.dma_start(ident_sb[:], ident_d.ap())

                # persistent state: bf16 [x; h] stacks + f32 copies for Y/query
                xh0b = p1r.tile([128, 12, BL], BF16, tag="xh0b")  # emb | ctxT | h0T
                xh1b = p1r.tile([128, 8, BL], BF16, tag="xh1b")   # h0nT | h1T
                ctT_sb = p1r.tile([128, 4, BL], F32, tag="ctTsb")
                h1T_sb = p1r.tile([128, 4, BL], F32, tag="h1Tsb")
                h0_bt = p1r.tile([BL, H], F32, tag="h0bt")
                h1_bt = p1r.tile([BL, H], F32, tag="h1bt")
                sc_sb = p1r.tile([128, T], F32, tag="scsb")
                att = p1r.tile([128, T], F32, tag="att")
                att2 = p1r.tile([128, T], F32, tag="att2")
                ctx_sb = p1r.tile([128, E], F32, tag="ctxsb")
                nc.gpsimd.memset(sc_sb[:], 0.0)
                nc.gpsimd.memset(att[:], 0.0)
                nc.gpsimd.memset(att2[:], 0.0)
                nc.gpsimd.memset(ctx_sb[:], 0.0)
                nc.gpsimd.memset(xh0b[:], 0.0)
                nc.gpsimd.memset(xh1b[:], 0.0)
                nc.gpsimd.memset(h0_bt[:], 0.0)
                nc.gpsimd.memset(h1_bt[:], 0.0)

                for u in range(u_steps):
                    # qWT[e, b] = W_attn.T @ q (q = emb at u=0 else h1)
                    qwt_ps = p1p.tile([128, 4, BL], F32, tag="pA")
                    q_rhs = embT_sb[:, :, 0, :] if u == 0 else h1T_sb[:]
                    for e4 in range(4):
                        for h4 in range(4):
                            nc.tensor.matmul(
                                qwt_ps[:, e4, :],
                                wattn_sb[:, h4, e4 * 128:(e4 + 1) * 128],
                                q_rhs[:, h4, :],
                                start=(h4 == 0), stop=(h4 == 3),
                            )
                    qwt_sb = p1s.tile([128, 4, BL], F32R, tag="qwt")
                    nc.scalar.copy(qwt_sb[:], qwt_ps[:])

                    # scores[b, t] = sum_e qW[b, e] * encE[b, e, t]
                    # each batch in its own PSUM bank at partition 0
                    sc_ps = [p1q.tile([1, T], F32, tag=f"pB{b}", name=f"sc_ps{b}")
                             for b in range(BL)]
                    for b in range(BL):
                        for e4 in range(4):
                            nc.tensor.matmul(
                                sc_ps[b][0:1, :],
                                qwt_sb[:, e4, b:b + 1],
                                encE_sb[:, e4, b, :],
                                start=(e4 == 0), stop=(e4 == 3),
                            )

                    # gather to rows 32b of the persistent tile, fusing the mask
                    for b in range(BL):
                        nc.vector.tensor_tensor(sc_sb[32 * b:32 * b + 1, :],
                                                sc_ps[b][0:1, :],
                                                mask_sb[32 * b:32 * b + 1, :],
                                                op=ALU.add)
                    mx = p1s.tile([128, 1], F32, tag="mx")
                    nc.vector.reduce_max(mx[:], sc_sb[:], axis=AX.X)
                    nmx = p1s.tile([128, 1], F32, tag="nmx")
                    nc.vector.tensor_scalar_mul(nmx[:], mx[:], -1.0)
                    ssum = p1s.tile([128, 1], F32, tag="ssum")
                    nc.scalar.activation(att[:], sc_sb[:], ACTF.Exp,
                                         bias=nmx[:], scale=1.0, accum_out=ssum[:])
                    nc.vector.tensor_scalar_add(ssum[:], ssum[:], 1e-30)
                    rec = p1s.tile([128, 1], F32, tag="rec")
                    nc.vector.reciprocal(rec[:], ssum[:])
                    nc.vector.tensor_scalar_mul(att2[:], att[:], rec[:])

                    # attT via full PE transpose; batch b ends up in column 32b
                    atT_ps = p1p.tile([128, 4, 128], F32, tag="pA")
                    for tc4 in range(4):
                        nc.tensor.transpose(
                            atT_ps[:, tc4, :],
                            att2[:, tc4 * 128:(tc4 + 1) * 128],
                            ident_sb[:],
                        )
                    atT_sb = p1s.tile([128, 4, 128], F32R, tag="atT")
                    nc.scalar.copy(atT_sb[:], atT_ps[:])

                    # ctx[b, e] = sum_t att[b, t] * encT[b, t, e]
                    ctx_ps = [p1q.tile([1, E], F32, tag=f"pB{b}", name=f"ctx_ps{b}")
                              for b in range(BL)]
                    for b in range(BL):
                        for tc4 in range(4):
                            nc.tensor.matmul(
                                ctx_ps[b][0:1, :],
                                atT_sb[:, tc4, 32 * b:32 * b + 1],
                                encT_sb[:, tc4, b, :],
                                start=(tc4 == 0), stop=(tc4 == 3),
                            )
                    for b in range(BL):
                        nc.scalar.copy(ctx_sb[32 * b:32 * b + 1, :],
                                       ctx_ps[b][0:1, :])

                    # ctxT -> xh0 chunks 4:8; emb_u -> chunks 0:4
                    ctT_ps = p1p.tile([128, 4, 128], F32, tag="pA")
                    for c4 in range(4):
                        nc.tensor.transpose(
                            ctT_ps[:, c4, :],
                            ctx_sb[:, c4 * 128:(c4 + 1) * 128],
                            ident_sb[:],
                        )
                    nc.vector.tensor_copy(ctT_sb[:], ctT_ps[:, :, 0:128:32])
                    nc.vector.tensor_copy(xh0b[:, 4:8, :], ctT_ps[:, :, 0:128:32])
                    nc.vector.tensor_copy(xh0b[:, 0:4, :], embT_sb[:, :, u, :])

                    # ---- GRU layer 0 ----
                    grz_ps = p1q.tile([BL, 2 * H], F32, tag="pC")
                    for j in range(12):
                        for n2 in range(2):
                            nc.tensor.matmul(
                                grz_ps[:, n2 * 512:(n2 + 1) * 512],
                                xh0b[:, j, :],
                                wrz0_sb[:, j, n2 * 512:(n2 + 1) * 512],
                                start=(j == 0), stop=(j == 11),
                            )
                    gni_ps = p1q.tile([BL, H], F32, tag="pB0")
                    for j in range(8):
                        nc.tensor.matmul(gni_ps[:], xh0b[:, j, :], wn0i_sb[:, j, :],
                                         start=(j == 0), stop=(j == 7))
                    gnh_ps = p1q.tile([BL, H], F32, tag="pB1")
                    for j in range(4):
                        nc.tensor.matmul(gnh_ps[:], xh0b[:, 8 + j, :], wn0h_sb[:, j, :],
                                         start=(j == 0), stop=(j == 3))

                    tmp_rz = p1g.tile([BL, 2 * H], F32, tag="trz")
                    nc.vector.tensor_tensor(tmp_rz[:], grz_ps[:],
                                            brz0_sb[:], op=ALU.add)
                    sig = p1g.tile([BL, 2 * H], F32, tag="sig")
                    nc.scalar.activation(sig[:], tmp_rz[:], ACTF.Sigmoid)
                    tn = p1g.tile([BL, H], F32, tag="tn")
                    nc.vector.tensor_tensor(tn[:], gnh_ps[:],
                                            bn0h_sb[:], op=ALU.add)
                    nc.vector.tensor_tensor(tn[:], tn[:], sig[:, 0:H], op=ALU.mult)
                    nc.vector.tensor_tensor(tn[:], tn[:], gni_ps[:], op=ALU.add)
                    nc.vector.tensor_tensor(tn[:], tn[:],
                                            bn0i_sb[:], op=ALU.add)
                    n0 = p1g.tile([BL, H], F32, tag="n0")
                    nc.scalar.activation(n0[:], tn[:], ACTF.Tanh)
                    d0 = p1g.tile([BL, H], F32, tag="d0")
                    nc.vector.tensor_tensor(d0[:], h0_bt[:], n0[:], op=ALU.subtract)
                    nc.vector.tensor_tensor(d0[:], sig[:, H:2 * H], d0[:], op=ALU.mult)
                    nc.vector.tensor_tensor(h0_bt[:], n0[:], d0[:], op=ALU.add)

                    h0T_ps = p1p.tile([128, 4, BL], F32, tag="pA")
                    for c4 in range(4):
                        nc.tensor.transpose(
                            h0T_ps[:, c4, :],
                            h0_bt[0:BL, c4 * 128:(c4 + 1) * 128],
                            ident_sb[0:BL, 0:BL],
                        )
                    nc.vector.tensor_copy(xh1b[:, 0:4, :], h0T_ps[:])
                    nc.vector.tensor_copy(xh0b[:, 8:12, :], h0T_ps[:])

                    # ---- GRU layer 1 ----
                    grz1_ps = p1q.tile([BL, 2 * H], F32, tag="pC")
                    for j in range(8):
                        for n2 in range(2):
                            nc.tensor.matmul(
                                grz1_ps[:, n2 * 512:(n2 + 1) * 512],
                                xh1b[:, j, :],
                                wrz1_sb[:, j, n2 * 512:(n2 + 1) * 512],
                                start=(j == 0), stop=(j == 7),
                            )
                    gni1_ps = p1q.tile([BL, H], F32, tag="pB2")
                    for j in range(4):
                        nc.tensor.matmul(gni1_ps[:], xh1b[:, j, :], wn1i_sb[:, j, :],
                                         start=(j == 0), stop=(j == 3))
                    gnh1_ps = p1q.tile([BL, H], F32, tag="pB3")
                    for j in range(4):
                        nc.tensor.matmul(gnh1_ps[:], xh1b[:, 4 + j, :], wn1h_sb[:, j, :],
                                         start=(j == 0), stop=(j == 3))

                    tmp_rz1 = p1g.tile([BL, 2 * H], F32, tag="trz")
                    nc.vector.tensor_tensor(tmp_rz1[:], grz1_ps[:],
                                            brz1_sb[:], op=ALU.add)
                    sig1 = p1g.tile([BL, 2 * H], F32, tag="sig")
                    nc.scalar.activation(sig1[:], tmp_rz1[:], ACTF.Sigmoid)
                    tn1 = p1g.tile([BL, H], F32, tag="tn")
                    nc.vector.tensor_tensor(tn1[:], gnh1_ps[:],
                                            bn1h_sb[:], op=ALU.add)
                    nc.vector.tensor_tensor(tn1[:], tn1[:], sig1[:, 0:H], op=ALU.mult)
                    nc.vector.tensor_tensor(tn1[:], tn1[:], gni1_ps[:], op=ALU.add)
                    nc.vector.tensor_tensor(tn1[:], tn1[:],
                                            bn1i_sb[:], op=ALU.add)
                    n1 = p1g.tile([BL, H], F32, tag="n0")
                    nc.scalar.activation(n1[:], tn1[:], ACTF.Tanh)
                    d1 = p1g.tile([BL, H], F32, tag="d0")
                    nc.vector.tensor_tensor(d1[:], h1_bt[:], n1[:], op=ALU.subtract)
                    nc.vector.tensor_tensor(d1[:], sig1[:, H:2 * H], d1[:], op=ALU.mult)
                    nc.vector.tensor_tensor(h1_bt[:], n1[:], d1[:], op=ALU.add)

                    h1T_ps = p1p.tile([128, 4, BL], F32, tag="pA")
                    for c4 in range(4):
                        nc.tensor.transpose(
                            h1T_ps[:, c4, :],
                            h1_bt[0:BL, c4 * 128:(c4 + 1) * 128],
                            ident_sb[0:BL, 0:BL],
                        )
                    nc.vector.tensor_copy(h1T_sb[:], h1T_ps[:])
                    nc.vector.tensor_copy(xh1b[:, 4:8, :], h1T_ps[:])

                    # stage Y_u^T = [h1T; ctxT] to DRAM
                    nc.sync.dma_start(
                        ytl[0:4, :, u * BL:(u + 1) * BL].transpose([1, 0, 2]),
                        h1T_sb[:],
                    )
                    nc.sync.dma_start(
                        ytl[4:8, :, u * BL:(u + 1) * BL].transpose([1, 0, 2]),
                        ctT_sb[:],
                    )

            # ---- all-gather Y ----
            nc.gpsimd.collective_compute(
                "AllGather",
                ALU.bypass,
                ins=[ytl[:].opt()],
                outs=[yts[:].opt()],
                replica_groups=[list(range(NCORES))],
            )

            # ---- phase 2: vocab-sharded projection ----
            with (
                tc.tile_pool(name="p2r", bufs=1) as p2r,
                tc.tile_pool(name="p2s", bufs=2) as p2s,
                tc.tile_pool(name="p2o", bufs=4) as p2o,
                tc.tile_pool(name="p2p", bufs=4, space="PSUM") as p2p,
            ):
                yts_sb = p2r.tile([128, 64, UBL], F32R, tag="ytssb")
                nc.sync.dma_start(yts_sb[:], yts[:].transpose([1, 0, 2]).bitcast(F32R))

                for vt in range(VSP // 128):
                    wt = p2s.tile([128, 8, 128], F32R, tag="wt")
                    nc.sync.dma_start(
                        wt[:], woutT_d.ap()[:, :, vt * 128:(vt + 1) * 128].transpose([1, 0, 2])
                    )
                    for rp in range(4):
                        ps = p2p.tile([128, 512], F32, tag="p2")
                        for rh in range(2):
                            r = rp * 2 + rh
                            off = rh * 256
                            for kc in range(8):
                                nc.tensor.matmul(
                                    ps[:, off:off + 256],
                                    wt[:, kc, :],
                                    yts_sb[:, r * 8 + kc, :],
                                    start=(kc == 0), stop=(kc == 7),
                                )
                        ob = p2o.tile([128, 512], F32, tag="ob")
                        nc.vector.tensor_scalar_add(ob[:], ps[:], bout_sb[:, vt:vt + 1])
                        nc.sync.dma_start(
                            out_d.ap()[vt * 128:(vt + 1) * 128, rp * 512:(rp + 1) * 512],
                            ob[:],
                        )

    nc.finalize()
    return nc


_NC_CACHE = None


def _get_nc():
    global _NC_CACHE
    if _NC_CACHE is None:
        _NC_CACHE = build_nc()
    return _NC_CACHE


def make_in_maps(inputs):
    f32 = np.float32
    enc = np.ascontiguousarray(np.asarray(inputs["encoder_out"], f32))
    lens = np.asarray(inputs["encoder_lens"])
    dec = np.asarray(inputs["decoder_in"])
    emb_table = np.asarray(inputs["emb_table"], f32)
    W_attn = np.asarray(inputs["W_attn"], f32)
    W_ih0 = np.asarray(inputs["W_ih0"], f32)
    W_hh0 = np.asarray(inputs["W_hh0"], f32)
    b_ih0 = np.asarray(inputs["b_ih0"], f32)
    b_hh0 = np.asarray(inputs["b_hh0"], f32)
    W_ih1 = np.asarray(inputs["W_ih1"], f32)
    W_hh1 = np.asarray(inputs["W_hh1"], f32)
    b_ih1 = np.asarray(inputs["b_ih1"], f32)
    b_hh1 = np.asarray(inputs["b_hh1"], f32)
    W_out = np.asarray(inputs["W_out"], f32)
    b_out = np.asarray(inputs["b_out"], f32)

    bf = lambda x: np.ascontiguousarray(x).astype(ml_dtypes.bfloat16)
    chunk = lambda x: np.ascontiguousarray(
        x.reshape(x.shape[0] // 128, 128, x.shape[1]))

    embedded = emb_table[dec]                       # [B, U, H]
    mask = np.where(
        np.arange(T)[None, :] >= np.asarray(lens, np.int64)[:, None],
        f32(-1e30), f32(0.0))                       # [B, T]


    wrz0 = chunk(bf(np.concatenate([W_ih0.T[:, :2 * H], W_hh0.T[:, :2 * H]], 0)))
    wn0i = chunk(bf(W_ih0.T[:, 2 * H:]))
    wn0h = chunk(bf(W_hh0.T[:, 2 * H:]))
    wrz1 = chunk(bf(np.concatenate([W_ih1.T[:, :2 * H], W_hh1.T[:, :2 * H]], 0)))
    wn1i = chunk(bf(W_ih1.T[:, 2 * H:]))
    wn1h = chunk(bf(W_hh1.T[:, 2 * H:]))
    bcast = lambda v: np.ascontiguousarray(
        np.broadcast_to(v.reshape(1, -1), (BL, v.shape[0])).astype(f32))
    brz0 = bcast(b_ih0[:2 * H] + b_hh0[:2 * H])
    bn0i = bcast(b_ih0[2 * H:])
    bn0h = bcast(b_hh0[2 * H:])
    brz1 = bcast(b_ih1[:2 * H] + b_hh1[:2 * H])
    bn1i = bcast(b_ih1[2 * H:])
    bn1h = bcast(b_hh1[2 * H:])
    ident = np.eye(128, dtype=f32)

    in_maps = []
    for c in range(NCORES):
        bs = slice(BL * c, BL * (c + 1))
        woutT = np.zeros((H + E, VSP), f32)
        woutT[:, :VS] = W_out[VS * c:VS * (c + 1)].T
        boutp = np.zeros((VSP,), f32)
        boutp[:VS] = b_out[VS * c:VS * (c + 1)]
        boutp = np.ascontiguousarray(boutp.reshape(VSP // 128, 128).T)
        in_maps.append({
            "encE": np.ascontiguousarray(enc[bs].transpose(0, 2, 1)),
            "encT": np.ascontiguousarray(enc[bs]),
            "embT": np.ascontiguousarray(embedded[bs].transpose(1, 2, 0)),
            "mask": _expand_mask(mask[bs]),
            "wattn": W_attn,
            "wrz0": wrz0, "wn0i": wn0i, "wn0h": wn0h,
            "wrz1": wrz1, "wn1i": wn1i, "wn1h": wn1h,
            "brz0": brz0, "bn0i": bn0i, "bn0h": bn0h,
            "brz1": brz1, "bn1i": bn1i, "bn1h": bn1h,
            "woutT": chunk(woutT),
            "bout": boutp,
            "ident": ident,
        })
    return in_maps


def _expand_mask(mask_bl):
    m = np.full((128, T), np.float32(-1e30))
    for b in range(mask_bl.shape[0]):
        m[32 * b, :] = mask_bl[b]
    return np.ascontiguousarray(m)


def assemble_output(results):
    logits = np.zeros((B, U, V), np.float32)
    for c in range(NCORES):
        o = results[c]["out"][:VS]                     # [4000, 2048]
        o = o.reshape(VS, NCORES, U, BL).transpose(1, 3, 2, 0)  # [r, bl, U, VS]
        logits[:, :, VS * c:VS * (c + 1)] = o.reshape(B, U, VS)
    return logits


def kernel(**inputs):
    nc = _get_nc()
    in_maps = make_in_maps(inputs)
    res = run_bass_kernel_spmd(nc, in_maps, core_ids=list(range(NCORES)))
    return assemble_output(res.results)


if __name__ == "__main__":
    nc = build_nc()
    print("built OK")



# revision 11
# speedup vs baseline: 2.7350x; 2.7350x over previous
"""AttentionDecoder Trainium2 kernel (8 NeuronCores).

Strategy (v2):
  - Batch-shard everything: core c owns batches [4c, 4c+4). No collectives.
  - enc_proj = W_attn-projected encoder is precomputed once per core, and the
    embedding contribution to GRU-layer-0 gates (+ its biases) is precomputed
    for all 64 steps in one GEMM, so the per-step recurrence only contracts
    ctx/h terms.
  - The recurrence keeps every activation in [feature-on-partitions, batch]
    layout. GRU matmuls run with the weight chunk as the 128x128 stationary
    operand (bf16, fast-weight-load) and the 4-wide activations moving, so
    gates land as [gate_dim, batch] and all gate math is short-free-dim
    DVE/ACT ops. sigmoid(x) = 0.5*tanh(x/2)+0.5 keeps the scalar engine on
    one activation table (exp+tanh) forever.
  - Attention scores for the 4 batches accumulate into one PSUM bank at
    partitions {0,32,64,96}; the length mask joins the accumulation as a
    1-row matmul issued a step early; softmax skips max-subtraction and the
    1/sum normalization is folded into the ctx PSUM->SBUF copy as a
    per-partition scale.
  - Y^T = [h1; ctx] accumulates in SBUF in bf16. Phase 2 computes the full
    vocab for the local 4 batches, streaming W_out^T (bf16) from HBM in
    2 MB super-tiles that double-buffer against the GEMM.
"""

import numpy as np
import ml_dtypes

import concourse.bass as bass
import concourse.bacc as bacc_mod
import concourse.mybir as mybir
from concourse import tile
from concourse.bass_utils import run_bass_kernel_spmd

B, T, U = 32, 512, 64
V, H, E = 32000, 512, 512
NCORES = 8
BL = B // NCORES          # local batches per core
NSUP = 32                 # phase-2 vocab super-tiles (8 x 128 vocab each)
VP = NSUP * 8 * 128       # padded vocab (32768)
UB_L = U * BL             # 256 local (u, b) columns

F32 = mybir.dt.float32
F32R = mybir.dt.float32r
BF16 = mybir.dt.bfloat16
AX = mybir.AxisListType
ALU = mybir.AluOpType
ACTF = mybir.ActivationFunctionType


def build_nc(u_steps=U, biases_zero=True):
    nc = bacc_mod.Bacc()

    encE_d = nc.declare_dram_parameter("encE", [128, 4, BL, T], BF16, isOutput=False)
    encT_d = nc.declare_dram_parameter("encT", [128, 4, BL, E], BF16, isOutput=False)
    embT_d = nc.declare_dram_parameter("embT", [128, 4, U, BL], BF16, isOutput=False)
    mask_d = nc.declare_dram_parameter("mask", [1, BL, T], BF16, isOutput=False)
    wattnT_d = nc.declare_dram_parameter("wattnT", [128, 4, 4, 128], BF16, isOutput=False)
    wemb0_d = nc.declare_dram_parameter("wemb0", [128, 4, 12, 128], BF16, isOutput=False)
    wrz0_d = nc.declare_dram_parameter("wrz0", [128, 8, 8, 128], BF16, isOutput=False)
    wn0i_d = nc.declare_dram_parameter("wn0i", [128, 4, 4, 128], BF16, isOutput=False)
    wn0h_d = nc.declare_dram_parameter("wn0h", [128, 4, 4, 128], BF16, isOutput=False)
    wrz1_d = nc.declare_dram_parameter("wrz1", [128, 8, 8, 128], BF16, isOutput=False)
    wn1i_d = nc.declare_dram_parameter("wn1i", [128, 4, 4, 128], BF16, isOutput=False)
    wn1h_d = nc.declare_dram_parameter("wn1h", [128, 4, 4, 128], BF16, isOutput=False)
    woutT_d = nc.declare_dram_parameter("woutT", [NSUP, 128, 8, 8, 128], BF16, isOutput=False)
    bout_d = nc.declare_dram_parameter("bout", [128, NSUP * 8], F32, isOutput=False)
    identb_d = nc.declare_dram_parameter("identb", [128, 128], BF16, isOutput=False)
    bias0_d = nc.declare_dram_parameter("bias0", [128, 12], F32, isOutput=False)
    brz1_d = nc.declare_dram_parameter("brz1", [128, 8, BL], F32, isOutput=False)
    bnh0_d = nc.declare_dram_parameter("bnh0", [128, 4, BL], F32, isOutput=False)
    bni1_d = nc.declare_dram_parameter("bni1", [128, 4, BL], F32, isOutput=False)
    bnh1_d = nc.declare_dram_parameter("bnh1", [128, 4, BL], F32, isOutput=False)
    out_d = nc.declare_dram_parameter("out", [NSUP, 128, 8, U, BL], F32, isOutput=True)

    with tile.TileContext(nc) as tc:
        with tc.tile_pool(name="res", bufs=1) as res:
            # ---- resident SBUF ----
            encT_sb = res.tile([128, 4, BL, E], BF16, tag="encT")
            nc.sync.dma_start(encT_sb[:], encT_d.ap())
            embT_sb = res.tile([128, 4, U, BL], BF16, tag="embT")
            nc.sync.dma_start(embT_sb[:], embT_d.ap())
            mask_sb = res.tile([1, BL, T], BF16, tag="mask")
            nc.sync.dma_start(mask_sb[:], mask_d.ap())
            wrz0_sb = res.tile([128, 8, 8, 128], BF16, tag="wrz0")
            nc.sync.dma_start(wrz0_sb[:], wrz0_d.ap())
            wn0i_sb = res.tile([128, 4, 4, 128], BF16, tag="wn0i")
            nc.sync.dma_start(wn0i_sb[:], wn0i_d.ap())
            wn0h_sb = res.tile([128, 4, 4, 128], BF16, tag="wn0h")
            nc.sync.dma_start(wn0h_sb[:], wn0h_d.ap())
            wrz1_sb = res.tile([128, 8, 8, 128], BF16, tag="wrz1")
            nc.sync.dma_start(wrz1_sb[:], wrz1_d.ap())
            wn1i_sb = res.tile([128, 4, 4, 128], BF16, tag="wn1i")
            nc.sync.dma_start(wn1i_sb[:], wn1i_d.ap())
            wn1h_sb = res.tile([128, 4, 4, 128], BF16, tag="wn1h")
            nc.sync.dma_start(wn1h_sb[:], wn1h_d.ap())
            identb_sb = res.tile([128, 128], BF16, tag="identb")
            nc.sync.dma_start(identb_sb[:], identb_d.ap())
            bout_sb = res.tile([128, NSUP * 8], F32, tag="bout")
            nc.sync.dma_start(bout_sb[:], bout_d.ap())
            if not biases_zero:
                bias0_sb = res.tile([128, 12], F32, tag="bias0")
                nc.sync.dma_start(bias0_sb[:], bias0_d.ap())
                brz1_sb = res.tile([128, 8, BL], F32, tag="brz1")
                nc.sync.dma_start(brz1_sb[:], brz1_d.ap())
                bnh0_sb = res.tile([128, 4, BL], F32, tag="bnh0")
                nc.sync.dma_start(bnh0_sb[:], bnh0_d.ap())
                bni1_sb = res.tile([128, 4, BL], F32, tag="bni1")
                nc.sync.dma_start(bni1_sb[:], bni1_d.ap())
                bnh1_sb = res.tile([128, 4, BL], F32, tag="bnh1")
                nc.sync.dma_start(bnh1_sb[:], bnh1_d.ap())

            encP_sb = res.tile([128, 4, BL, T], BF16, tag="encP")
            embW0_sb = res.tile([128, 12, U, BL], F32, tag="embW0")
            yT_sb = res.tile([128, 8, U, BL], BF16, tag="yT")

            # persistent recurrence state
            h0f = res.tile([128, 4, BL], F32, tag="h0f")
            h1f = res.tile([128, 4, BL], F32, tag="h1f")
            h0b = res.tile([128, 4, BL], BF16, tag="h0b")
            zero_sb = res.tile([128, 4, BL], BF16, tag="zero")
            nc.gpsimd.memset(h0f[:], 0.0)
            nc.gpsimd.memset(h1f[:], 0.0)
            nc.gpsimd.memset(h0b[:], 0.0)
            nc.gpsimd.memset(zero_sb[:], 0.0)

            # ---- setup: encP = W_attn^T-projected enc; embW0 = Wih0_emb @ emb ----
            with (
                tc.tile_pool(name="su", bufs=1) as su,
                tc.tile_pool(name="sup", bufs=2, space="PSUM") as sup,
            ):
                encE_sb = su.tile([128, 4, BL, T], BF16, tag="encE")
                nc.sync.dma_start(encE_sb[:], encE_d.ap())
                wattnT_sb = su.tile([128, 4, 4, 128], BF16, tag="wattnT")
                nc.sync.dma_start(wattnT_sb[:], wattnT_d.ap())
                wemb0_sb = su.tile([128, 4, 12, 128], BF16, tag="wemb0")
                nc.sync.dma_start(wemb0_sb[:], wemb0_d.ap())

                for m in range(12):
                    ew_ps = sup.tile([128, U * BL], F32, tag="ewps")
                    for kc in range(4):
                        nc.tensor.matmul(
                            ew_ps[:],
                            wemb0_sb[:, kc, m, :],
                            embT_sb[:, kc, :, :],
                            start=(kc == 0), stop=(kc == 3),
                        )
                    if biases_zero:
                        if m % 2 == 0:
                            nc.vector.tensor_copy(embW0_sb[:, m, :, :], ew_ps[:])
                        else:
                            nc.scalar.copy(embW0_sb[:, m, :, :], ew_ps[:])
                    else:
                        nc.scalar.activation(embW0_sb[:, m, :, :], ew_ps[:],
                                             ACTF.Identity, bias=bias0_sb[:, m:m + 1])

                for b in range(BL):
                    for hc in range(4):
                        ep_ps = sup.tile([128, T], F32, tag="epps")
                        for ec in range(4):
                            nc.tensor.matmul(
                                ep_ps[:],
                                wattnT_sb[:, ec, hc, :],
                                encE_sb[:, ec, b, :],
                                start=(ec == 0), stop=(ec == 3),
                            )
                        if (b + hc) % 2 == 0:
                            nc.vector.tensor_copy(encP_sb[:, hc, b, :], ep_ps[:])
                        else:
                            nc.scalar.copy(encP_sb[:, hc, b, :], ep_ps[:])

            # ---- phase 1: recurrence ----
            with (
                tc.tile_pool(name="p1s", bufs=1) as p1s,
                tc.tile_pool(name="p1p", bufs=1, space="PSUM") as p1p,
            ):
                # persistent PSUM tiles; scores rotate on u parity for the
                # mask pre-accumulation trick
                sc_ps = [p1p.tile([128, T], F32, tag=f"sc{i}", name=f"sc{i}")
                         for i in range(2)]
                atT_ps = p1p.tile([128, 4, 128], BF16, tag="atT")
                ctx_ps = p1p.tile([128, E], F32, tag="ctx")
                ctT_ps = p1p.tile([128, 4, 128], BF16, tag="ctT")
                gates0_ps = p1p.tile([128, 16, BL], F32, tag="gates0")
                gates1_ps = p1p.tile([128, 16, BL], F32, tag="gates1")

                att_sb = p1s.tile([128, T], BF16, tag="att")
                ssum = p1s.tile([128, 1], F32, tag="ssum")
                rec = p1s.tile([128, 1], F32, tag="rec")
                atT_sb = p1s.tile([128, 4, BL], BF16, tag="atTsb")
                ctx_sb = p1s.tile([128, E], BF16, tag="ctxsb")
                g0_sb = p1s.tile([128, 8, BL], F32, tag="g0")
                t0_sb = p1s.tile([128, 8, BL], F32, tag="t0")
                ni0_sb = p1s.tile([128, 4, BL], F32, tag="ni0")
                a0_sb = p1s.tile([128, 4, BL], F32, tag="a0")
                np0_sb = p1s.tile([128, 4, BL], F32, tag="np0")
                n0_sb = p1s.tile([128, 4, BL], F32, tag="n0")
                d0_sb = p1s.tile([128, 4, BL], F32, tag="d0")
                g1_sb = p1s.tile([128, 8, BL], F32, tag="g1")
                t1_sb = p1s.tile([128, 8, BL], F32, tag="t1")
                a1_sb = p1s.tile([128, 4, BL], F32, tag="a1")
                np1_sb = p1s.tile([128, 4, BL], F32, tag="np1")
                n1_sb = p1s.tile([128, 4, BL], F32, tag="n1")
                d1_sb = p1s.tile([128, 4, BL], F32, tag="d1")
                ones_sb = p1s.tile([1, 1], BF16, tag="ones")
                ones128_sb = p1s.tile([1, 128], BF16, tag="ones128")
                zrow_sb = p1s.tile([1, T], BF16, tag="zrow")
                nc.gpsimd.memset(ones_sb[:], 1.0)
                nc.gpsimd.memset(ones128_sb[:], 1.0)
                nc.gpsimd.memset(zrow_sb[:], 0.0)

                # one-time init: write every partition row of the score/ctx
                # banks so never-again-written rows hold 0, not pre-kernel
                # garbage (exp/transpose would otherwise see inf/NaN there).
                for i in range(2):
                    nc.tensor.matmul(
                        sc_ps[i][:, :], ones128_sb[:], zrow_sb[:],
                        start=True, stop=False, skip_group_check=True,
                    )
                nc.tensor.matmul(
                    ctx_ps[:, :], ones128_sb[:], zrow_sb[:, 0:E],
                    start=True, stop=False, skip_group_check=True,
                )

                # mask pre-accumulation for u=0
                for b in range(BL):
                    nc.tensor.matmul(
                        sc_ps[0][32 * b:32 * b + 1, :],
                        ones_sb[:], mask_sb[:, b, :],
                        start=True, stop=False, skip_group_check=True,
                        tile_position=(0, 32 * b),
                    )

                for u in range(u_steps):
                    cur = sc_ps[u % 2]
                    nxt = sc_ps[(u + 1) % 2]

                    # scores[b, t] += sum_h q[h, b] * encP[b][h, t]
                    for b in range(BL):
                        for kc in range(4):
                            lhs = (embT_sb[:, kc, 0, b:b + 1] if u == 0
                                   else yT_sb[:, kc, u - 1, b:b + 1])
                            nc.tensor.matmul(
                                cur[32 * b:32 * b + 1, :],
                                lhs,
                                encP_sb[:, kc, b, :],
                                start=False, stop=(kc == 3),
                                skip_group_check=True,
                                tile_position=(0, 32 * b),
                            )

                    # mask pre-accumulation for u+1 (group START for nxt)
                    if u + 1 < u_steps:
                        for b in range(BL):
                            nc.tensor.matmul(
                                nxt[32 * b:32 * b + 1, :],
                                ones_sb[:], mask_sb[:, b, :],
                                start=True, stop=False, skip_group_check=True,
                                tile_position=(0, 32 * b),
                            )

                    # GRU0/GRU1 h-dependent contractions (ready at step start)
                    def h0rhs(k):
                        return zero_sb[:, k, :] if u == 0 else h0b[:, k, :]

                    def h1rhs(k):
                        return (zero_sb[:, k, :] if u == 0
                                else yT_sb[:, k, u - 1, :])

                    for m in range(4):
                        for k in range(4):
                            nc.tensor.matmul(
                                gates0_ps[:, 12 + m, :],
                                wn0h_sb[:, k, m, :],
                                h0rhs(k),
                                start=(k == 0), stop=(k == 3),
                            )
                    for m in range(8):
                        for k in range(4):
                            nc.tensor.matmul(
                                gates0_ps[:, m, :],
                                wrz0_sb[:, k, m, :],
                                h0rhs(k),
                                start=(k == 0), stop=False,
                            )

                    # softmax (no max-subtract; mask rows are -1e30)
                    nc.scalar.activation(att_sb[:], cur[:], ACTF.Exp,
                                         accum_out=ssum[:])
                    nc.vector.reciprocal(rec[:], ssum[:])

                    # attT: batch b sits in column 32b; keep those columns
                    for tc4 in range(4):
                        nc.tensor.transpose(
                            atT_ps[:, tc4, :],
                            att_sb[:, tc4 * 128:(tc4 + 1) * 128],
                            identb_sb[:],
                        )
                    nc.vector.tensor_copy(atT_sb[:], atT_ps[:, :, 0:128:32])

                    # ctx[b, e] += att[b, t] * encT[b][t, e]
                    for b in range(BL):
                        for tc4 in range(4):
                            nc.tensor.matmul(
                                ctx_ps[32 * b:32 * b + 1, :],
                                atT_sb[:, tc4, b:b + 1],
                                encT_sb[:, tc4, b, :],
                                start=(tc4 == 0), stop=(tc4 == 3),
                                skip_group_check=True,
                                tile_position=(0, 32 * b),
                            )

                    # GRU1 h1-dependent contractions fill the ctx gap
                    for m in range(4):
                        for k in range(4):
                            nc.tensor.matmul(
                                gates1_ps[:, 12 + m, :],
                                wn1h_sb[:, k, m, :],
                                h1rhs(k),
                                start=(k == 0), stop=(k == 3),
                            )
                    for m in range(8):
                        for k in range(4):
                            nc.tensor.matmul(
                                gates1_ps[:, m, :],
                                wrz1_sb[:, k, m, :],
                                h1rhs(k),
                                start=(k == 0), stop=False,
                            )

                    # ctx normalize-on-copy (scale = 1/sum per batch row)
                    nc.scalar.activation(ctx_sb[:], ctx_ps[:], ACTF.Copy,
                                         scale=rec[:])
                    for ec in range(4):
                        nc.tensor.transpose(
                            ctT_ps[:, ec, :],
                            ctx_sb[:, ec * 128:(ec + 1) * 128],
                            identb_sb[:],
                        )
                    nc.vector.tensor_copy(yT_sb[:, 4:8, u, :],
                                          ctT_ps[:, :, 0:128:32])

                    # GRU0 ctx-dependent contractions
                    for m in range(8):
                        for k in range(4):
                            nc.tensor.matmul(
                                gates0_ps[:, m, :],
                                wrz0_sb[:, 4 + k, m, :],
                                yT_sb[:, 4 + k, u, :],
                                start=False, stop=(k == 3),
                            )
                    for m in range(4):
                        for k in range(4):
                            nc.tensor.matmul(
                                gates0_ps[:, 8 + m, :],
                                wn0i_sb[:, k, m, :],
                                yT_sb[:, 4 + k, u, :],
                                start=(k == 0), stop=(k == 3),
                            )

                    # ---- GRU0 gate math ([128, m, b] layout) ----
                    nc.vector.tensor_tensor(g0_sb[:], gates0_ps[:, 0:8, :],
                                            embW0_sb[:, 0:8, u, :], op=ALU.add)
                    nc.scalar.activation(t0_sb[:], g0_sb[:], ACTF.Tanh,
                                         scale=0.5)
                    nc.vector.tensor_tensor(ni0_sb[:], gates0_ps[:, 8:12, :],
                                            embW0_sb[:, 8:12, u, :], op=ALU.add)
                    if biases_zero:
                        nc.vector.scalar_tensor_tensor(
                            a0_sb[:], t0_sb[:, 0:4, :], 1.0,
                            gates0_ps[:, 12:16, :],
                            op0=ALU.add, op1=ALU.mult)
                    else:
                        nc.vector.tensor_tensor(a0_sb[:],
                                                gates0_ps[:, 12:16, :],
                                                bnh0_sb[:], op=ALU.add)
                        nc.vector.scalar_tensor_tensor(
                            a0_sb[:], t0_sb[:, 0:4, :], 1.0, a0_sb[:],
                            op0=ALU.add, op1=ALU.mult)
                    nc.vector.scalar_tensor_tensor(
                        np0_sb[:], a0_sb[:], 0.5, ni0_sb[:],
                        op0=ALU.mult, op1=ALU.add)
                    nc.scalar.activation(n0_sb[:], np0_sb[:], ACTF.Tanh)
                    nc.vector.tensor_tensor(d0_sb[:], h0f[:], n0_sb[:],
                                            op=ALU.subtract)
                    nc.vector.scalar_tensor_tensor(
                        d0_sb[:], t0_sb[:, 4:8, :], 1.0, d0_sb[:],
                        op0=ALU.add, op1=ALU.mult)
                    nc.vector.scalar_tensor_tensor(
                        h0f[:], d0_sb[:], 0.5, n0_sb[:],
                        op0=ALU.mult, op1=ALU.add)
                    nc.vector.tensor_copy(h0b[:], h0f[:])

                    # GRU1 h0n-dependent contractions
                    for m in range(8):
                        for k in range(4):
                            nc.tensor.matmul(
                                gates1_ps[:, m, :],
                                wrz1_sb[:, 4 + k, m, :],
                                h0b[:, k, :],
                                start=False, stop=(k == 3),
                            )
                    for m in range(4):
                        for k in range(4):
                            nc.tensor.matmul(
                                gates1_ps[:, 8 + m, :],
                                wn1i_sb[:, k, m, :],
                                h0b[:, k, :],
                                start=(k == 0), stop=(k == 3),
                            )

                    # ---- GRU1 gate math ----
                    if biases_zero:
                        nc.scalar.activation(t1_sb[:], gates1_ps[:, 0:8, :],
                                             ACTF.Tanh, scale=0.5)
                        nc.vector.scalar_tensor_tensor(
                            a1_sb[:], t1_sb[:, 0:4, :], 1.0,
                            gates1_ps[:, 12:16, :],
                            op0=ALU.add, op1=ALU.mult)
                        nc.vector.scalar_tensor_tensor(
                            np1_sb[:], a1_sb[:], 0.5, gates1_ps[:, 8:12, :],
                            op0=ALU.mult, op1=ALU.add)
                    else:
                        nc.vector.tensor_tensor(g1_sb[:], gates1_ps[:, 0:8, :],
                                                brz1_sb[:], op=ALU.add)
                        nc.scalar.activation(t1_sb[:], g1_sb[:], ACTF.Tanh,
                                             scale=0.5)
                        nc.vector.tensor_tensor(a1_sb[:],
                                                gates1_ps[:, 12:16, :],
                                                bnh1_sb[:], op=ALU.add)
                        nc.vector.scalar_tensor_tensor(
                            a1_sb[:], t1_sb[:, 0:4, :], 1.0, a1_sb[:],
                            op0=ALU.add, op1=ALU.mult)
                        nc.vector.tensor_tensor(np1_sb[:],
                                                gates1_ps[:, 8:12, :],
                                                bni1_sb[:], op=ALU.add)
                        nc.vector.scalar_tensor_tensor(
                            np1_sb[:], a1_sb[:], 0.5, np1_sb[:],
                            op0=ALU.mult, op1=ALU.add)
                    nc.scalar.activation(n1_sb[:], np1_sb[:], ACTF.Tanh)
                    nc.vector.tensor_tensor(d1_sb[:], h1f[:], n1_sb[:],
                                            op=ALU.subtract)
                    nc.vector.scalar_tensor_tensor(
                        d1_sb[:], t1_sb[:, 4:8, :], 1.0, d1_sb[:],
                        op0=ALU.add, op1=ALU.mult)
                    nc.vector.scalar_tensor_tensor(
                        h1f[:], d1_sb[:], 0.5, n1_sb[:],
                        op0=ALU.mult, op1=ALU.add)
                    nc.vector.tensor_copy(yT_sb[:, 0:4, u, :], h1f[:])

            # ---- phase 2: full-vocab projection for the local batches ----
            with (
                tc.tile_pool(name="p2w", bufs=2) as p2w,
                tc.tile_pool(name="p2o", bufs=2) as p2o,
                tc.tile_pool(name="p2p", bufs=4, space="PSUM") as p2p,
            ):
                for s in range(NSUP):
                    wt = p2w.tile([128, 8, 8, 128], BF16, tag="wt")
                    nc.sync.dma_start(wt[:], woutT_d.ap()[s])
                    ob = p2o.tile([128, 8, UB_L], F32, tag="ob")
                    for vc in range(8):
                        ps = p2p.tile([128, UB_L], F32, tag="p2")
                        for kc in range(8):
                            nc.tensor.matmul(
                                ps[:],
                                wt[:, vc, kc, :],
                                yT_sb[:, kc, :, :],
                                start=(kc == 0), stop=(kc == 7),
                            )
                        if vc % 2 == 0:
                            nc.scalar.activation(
                                ob[:, vc, :], ps[:], ACTF.Identity,
                                bias=bout_sb[:, s * 8 + vc:s * 8 + vc + 1])
                        else:
                            nc.vector.tensor_scalar_add(
                                ob[:, vc, :], ps[:],
                                bout_sb[:, s * 8 + vc:s * 8 + vc + 1])
                    nc.sync.dma_start(out_d.ap()[s], ob[:])

    nc.finalize()
    return nc


_NC_CACHE = {}


def _get_nc(biases_zero=True):
    if biases_zero not in _NC_CACHE:
        _NC_CACHE[biases_zero] = build_nc(biases_zero=biases_zero)
    return _NC_CACHE[biases_zero]


def make_in_maps(inputs):
    f32 = np.float32
    bf = ml_dtypes.bfloat16
    enc = np.asarray(inputs["encoder_out"], f32)
    lens = np.asarray(inputs["encoder_lens"]).astype(np.int64)
    dec = np.asarray(inputs["decoder_in"]).astype(np.int64)
    emb_table = np.asarray(inputs["emb_table"], f32)
    W_attn = np.asarray(inputs["W_attn"], f32)
    W_ih0 = np.asarray(inputs["W_ih0"], f32)
    W_hh0 = np.asarray(inputs["W_hh0"], f32)
    b_ih0 = np.asarray(inputs["b_ih0"], f32)
    b_hh0 = np.asarray(inputs["b_hh0"], f32)
    W_ih1 = np.asarray(inputs["W_ih1"], f32)
    W_hh1 = np.asarray(inputs["W_hh1"], f32)
    b_ih1 = np.asarray(inputs["b_ih1"], f32)
    b_hh1 = np.asarray(inputs["b_hh1"], f32)
    W_out = np.asarray(inputs["W_out"], f32)
    b_out = np.asarray(inputs["b_out"], f32)

    embedded = emb_table[dec]                       # [B, U, H]
    mask = np.where(
        np.arange(T)[None, :] >= lens[:, None],
        f32(-1e30), f32(0.0))                       # [B, T]

    def chunkT(w):
        # [K, M] weight -> lhsT chunks [128, kc, mc, 128] (bf16)
        K, M = w.shape
        return np.ascontiguousarray(
            w.reshape(K // 128, 128, M // 128, 128).transpose(1, 0, 2, 3)
        ).astype(bf)

    # per-step GRU lhsT chunk tables; k-order: h-part first, then ctx/x-part
    wrz0 = np.concatenate([W_hh0[0:1024].T, W_ih0[0:1024, 512:1024].T], 0)
    wrz0 = chunkT(wrz0)                             # [128, 8, 8, 128]
    wn0i = chunkT(W_ih0[1024:1536, 512:1024].T)
    wn0h = chunkT(W_hh0[1024:1536].T)
    wrz1 = np.concatenate([W_hh1[0:1024].T, W_ih1[0:1024].T], 0)
    wrz1 = chunkT(wrz1)
    wn1i = chunkT(W_ih1[1024:1536].T)
    wn1h = chunkT(W_hh1[1024:1536].T)
    wemb0 = chunkT(W_ih0[:, 0:512].T)               # [128, 4, 12, 128]
    wattnT = chunkT(W_attn.T)                       # [128, 4ec, 4hc, 128]

    Wp = np.zeros((VP, 1024), f32)
    Wp[:V] = W_out
    woutT = np.ascontiguousarray(
        Wp.reshape(NSUP, 8, 128, 8, 128).transpose(0, 4, 1, 3, 2)
    ).astype(bf)                                    # [32, 128k, 8vc, 8kc, 128v]
    bp = np.zeros((VP,), f32)
    bp[:V] = b_out
    bout_t = np.ascontiguousarray(bp.reshape(NSUP * 8, 128).T)

    # biases
    bias0 = np.zeros((128, 12), f32)                # embW0 bias (rz: ih+hh, n_i: ih)
    brz = (b_ih0[:1024] + b_hh0[:1024]).reshape(8, 128).T
    bias0[:, 0:8] = brz
    bias0[:, 8:12] = b_ih0[1024:1536].reshape(4, 128).T
    bcast = lambda v: np.ascontiguousarray(np.broadcast_to(
        v.reshape(v.shape[0] // 128, 128).T[:, :, None], (128, v.shape[0] // 128, BL)))
    brz1 = bcast(b_ih1[:1024] + b_hh1[:1024])
    bnh0 = bcast(b_hh0[1024:1536])
    bni1 = bcast(b_ih1[1024:1536])
    bnh1 = bcast(b_hh1[1024:1536])

    identb = np.eye(128, dtype=f32).astype(bf)

    in_maps = []
    for c in range(NCORES):
        bs = slice(BL * c, BL * (c + 1))
        encl = enc[bs]                              # [BL, T, E]
        encE = np.ascontiguousarray(
            encl.transpose(2, 0, 1).reshape(4, 128, BL, T).transpose(1, 0, 2, 3)
        ).astype(bf)                                # [128, 4ec, BL, T]
        encTt = np.ascontiguousarray(
            encl.transpose(1, 0, 2).reshape(4, 128, BL, E).transpose(1, 0, 2, 3)
        ).astype(bf)                                # [128, 4tc, BL, E]
        embT = np.ascontiguousarray(
            embedded[bs].transpose(2, 1, 0).reshape(4, 128, U, BL).transpose(1, 0, 2, 3)
        ).astype(bf)                                # [128, 4hc, U, BL]
        in_maps.append({
            "encE": encE,
            "encT": encTt,
            "embT": embT,
            "mask": np.ascontiguousarray(mask[bs][None, :, :]).astype(bf),
            "wattnT": wattnT,
            "wemb0": wemb0,
            "wrz0": wrz0, "wn0i": wn0i, "wn0h": wn0h,
            "wrz1": wrz1, "wn1i": wn1i, "wn1h": wn1h,
            "woutT": woutT,
            "bout": bout_t,
            "identb": identb,
            "bias0": bias0,
            "brz1": brz1, "bnh0": bnh0, "bni1": bni1, "bnh1": bnh1,
        })
    return in_maps


def assemble_output(results):
    logits = np.zeros((B, U, V), np.float32)
    for c in range(NCORES):
        o = results[c]["out"]                       # [32, 128, 8, U, BL]
        o = o.transpose(4, 3, 0, 2, 1).reshape(BL, U, VP)
        logits[BL * c:BL * (c + 1)] = o[:, :, :V]
    return logits


def kernel(**inputs):
    bz = all(
        float(np.abs(np.asarray(inputs[k])).max()) == 0.0
        for k in ("b_ih0", "b_hh0", "b_ih1", "b_hh1")
    )
    nc = _get_nc(biases_zero=bz)
    in_maps = make_in_maps(inputs)
    res = run_bass_kernel_spmd(nc, in_maps, core_ids=list(range(NCORES)))
    return assemble_output(res.results)


if __name__ == "__main__":
    nc = build_nc()
    print("built OK")


# revision 14
# speedup vs baseline: 2.8632x; 1.0469x over previous
"""AttentionDecoder Trainium2 kernel (8 NeuronCores).

Strategy (v2):
  - Batch-shard everything: core c owns batches [4c, 4c+4). No collectives.
  - enc_proj = W_attn-projected encoder is precomputed once per core, and the
    embedding contribution to GRU-layer-0 gates (+ its biases) is precomputed
    for all 64 steps in one GEMM, so the per-step recurrence only contracts
    ctx/h terms.
  - The recurrence keeps every activation in [feature-on-partitions, batch]
    layout. GRU matmuls run with the weight chunk as the 128x128 stationary
    operand (bf16, fast-weight-load) and the 4-wide activations moving, so
    gates land as [gate_dim, batch] and all gate math is short-free-dim
    DVE/ACT ops. sigmoid(x) = 0.5*tanh(x/2)+0.5 keeps the scalar engine on
    one activation table (exp+tanh) forever.
  - Attention scores for the 4 batches accumulate into one PSUM bank at
    partitions {0,32,64,96}; the length mask joins the accumulation as a
    1-row matmul issued a step early; softmax skips max-subtraction and the
    1/sum normalization is folded into the ctx PSUM->SBUF copy as a
    per-partition scale.
  - Y^T = [h1; ctx] accumulates in SBUF in bf16. Phase 2 computes the full
    vocab for the local 4 batches, streaming W_out^T (bf16) from HBM in
    2 MB super-tiles that double-buffer against the GEMM.
"""

import numpy as np
import ml_dtypes

import concourse.bass as bass
import concourse.bacc as bacc_mod
import concourse.mybir as mybir
from concourse import tile
from concourse.bass_utils import run_bass_kernel_spmd

B, T, U = 32, 512, 64
V, H, E = 32000, 512, 512
NCORES = 8
BL = B // NCORES          # local batches per core
NSUP = 32                 # phase-2 vocab super-tiles (8 x 128 vocab each)
VP = NSUP * 8 * 128       # padded vocab (32768)
UB_L = U * BL             # 256 local (u, b) columns

F32 = mybir.dt.float32
F32R = mybir.dt.float32r
BF16 = mybir.dt.bfloat16
AX = mybir.AxisListType
ALU = mybir.AluOpType
ACTF = mybir.ActivationFunctionType


def build_nc(u_steps=U, biases_zero=True):
    nc = bacc_mod.Bacc()

    encE_d = nc.declare_dram_parameter("encE", [128, 4, BL, T], BF16, isOutput=False)
    encT_d = nc.declare_dram_parameter("encT", [128, 4, BL, E], BF16, isOutput=False)
    embT_d = nc.declare_dram_parameter("embT", [128, 4, U, BL], BF16, isOutput=False)
    mask_d = nc.declare_dram_parameter("mask", [1, BL, T], BF16, isOutput=False)
    wattnT_d = nc.declare_dram_parameter("wattnT", [128, 4, 4, 128], BF16, isOutput=False)
    wemb0_d = nc.declare_dram_parameter("wemb0", [128, 4, 12, 128], BF16, isOutput=False)
    wrz0_d = nc.declare_dram_parameter("wrz0", [128, 8, 8, 128], BF16, isOutput=False)
    wn0i_d = nc.declare_dram_parameter("wn0i", [128, 4, 4, 128], BF16, isOutput=False)
    wn0h_d = nc.declare_dram_parameter("wn0h", [128, 4, 4, 128], BF16, isOutput=False)
    wrz1_d = nc.declare_dram_parameter("wrz1", [128, 8, 8, 128], BF16, isOutput=False)
    wn1i_d = nc.declare_dram_parameter("wn1i", [128, 4, 4, 128], BF16, isOutput=False)
    wn1h_d = nc.declare_dram_parameter("wn1h", [128, 4, 4, 128], BF16, isOutput=False)
    woutT_d = nc.declare_dram_parameter("woutT", [NSUP, 128, 8, 8, 128], BF16, isOutput=False)
    bout_d = nc.declare_dram_parameter("bout", [128, NSUP * 8], F32, isOutput=False)
    identb_d = nc.declare_dram_parameter("identb", [128, 128], BF16, isOutput=False)
    bias0_d = nc.declare_dram_parameter("bias0", [128, 12], F32, isOutput=False)
    brz1_d = nc.declare_dram_parameter("brz1", [128, 8, BL], F32, isOutput=False)
    bnh0_d = nc.declare_dram_parameter("bnh0", [128, 4, BL], F32, isOutput=False)
    bni1_d = nc.declare_dram_parameter("bni1", [128, 4, BL], F32, isOutput=False)
    bnh1_d = nc.declare_dram_parameter("bnh1", [128, 4, BL], F32, isOutput=False)
    out_d = nc.declare_dram_parameter("out", [NSUP, 128, 8, U, BL], BF16, isOutput=True)

    with tile.TileContext(nc) as tc:
        with tc.tile_pool(name="res", bufs=1) as res:
            # ---- resident SBUF ----
            encT_sb = res.tile([128, 4, BL, E], BF16, tag="encT")
            nc.sync.dma_start(encT_sb[:], encT_d.ap())
            embT_sb = res.tile([128, 4, U, BL], BF16, tag="embT")
            nc.sync.dma_start(embT_sb[:], embT_d.ap())
            mask_sb = res.tile([1, BL, T], BF16, tag="mask")
            nc.sync.dma_start(mask_sb[:], mask_d.ap())
            wrz0_sb = res.tile([128, 8, 8, 128], BF16, tag="wrz0")
            nc.sync.dma_start(wrz0_sb[:], wrz0_d.ap())
            wn0i_sb = res.tile([128, 4, 4, 128], BF16, tag="wn0i")
            nc.sync.dma_start(wn0i_sb[:], wn0i_d.ap())
            wn0h_sb = res.tile([128, 4, 4, 128], BF16, tag="wn0h")
            nc.sync.dma_start(wn0h_sb[:], wn0h_d.ap())
            wrz1_sb = res.tile([128, 8, 8, 128], BF16, tag="wrz1")
            nc.sync.dma_start(wrz1_sb[:], wrz1_d.ap())
            wn1i_sb = res.tile([128, 4, 4, 128], BF16, tag="wn1i")
            nc.sync.dma_start(wn1i_sb[:], wn1i_d.ap())
            wn1h_sb = res.tile([128, 4, 4, 128], BF16, tag="wn1h")
            nc.sync.dma_start(wn1h_sb[:], wn1h_d.ap())
            identb_sb = res.tile([128, 128], BF16, tag="identb")
            nc.sync.dma_start(identb_sb[:], identb_d.ap())
            bout_sb = res.tile([128, NSUP * 8], F32, tag="bout")
            nc.sync.dma_start(bout_sb[:], bout_d.ap())
            if not biases_zero:
                bias0_sb = res.tile([128, 12], F32, tag="bias0")
                nc.sync.dma_start(bias0_sb[:], bias0_d.ap())
                brz1_sb = res.tile([128, 8, BL], F32, tag="brz1")
                nc.sync.dma_start(brz1_sb[:], brz1_d.ap())
                bnh0_sb = res.tile([128, 4, BL], F32, tag="bnh0")
                nc.sync.dma_start(bnh0_sb[:], bnh0_d.ap())
                bni1_sb = res.tile([128, 4, BL], F32, tag="bni1")
                nc.sync.dma_start(bni1_sb[:], bni1_d.ap())
                bnh1_sb = res.tile([128, 4, BL], F32, tag="bnh1")
                nc.sync.dma_start(bnh1_sb[:], bnh1_d.ap())

            NPRE = 2
            wpre_sb = [res.tile([128, 8, 8, 128], BF16, tag=f"wpre{i}",
                                name=f"wpre{i}")
                       for i in range(NPRE)]
            for i in range(NPRE):
                nc.sync.dma_start(wpre_sb[i][:], woutT_d.ap()[i])

            encP_sb = res.tile([128, 4, BL, T], BF16, tag="encP")
            embW0_sb = res.tile([128, 12, U, BL], F32, tag="embW0")
            yT_sb = res.tile([128, 8, U, BL], BF16, tag="yT")

            # persistent recurrence state (h kept in bf16)
            h0b = res.tile([128, 4, BL], BF16, tag="h0b")
            zero_sb = res.tile([128, 4, BL], BF16, tag="zero")
            nc.gpsimd.memset(h0b[:], 0.0)
            nc.gpsimd.memset(zero_sb[:], 0.0)

            # ---- setup: encP = W_attn^T-projected enc; embW0 = Wih0_emb @ emb ----
            with (
                tc.tile_pool(name="su", bufs=1) as su,
                tc.tile_pool(name="sup", bufs=2, space="PSUM") as sup,
            ):
                encE_sb = su.tile([128, 4, BL, T], BF16, tag="encE")
                nc.sync.dma_start(encE_sb[:], encE_d.ap())
                wattnT_sb = su.tile([128, 4, 4, 128], BF16, tag="wattnT")
                nc.sync.dma_start(wattnT_sb[:], wattnT_d.ap())
                wemb0_sb = su.tile([128, 4, 12, 128], BF16, tag="wemb0")
                nc.sync.dma_start(wemb0_sb[:], wemb0_d.ap())

                for m in range(12):
                    ew_ps = sup.tile([128, U * BL], F32, tag="ewps")
                    for kc in range(4):
                        nc.tensor.matmul(
                            ew_ps[:],
                            wemb0_sb[:, kc, m, :],
                            embT_sb[:, kc, :, :],
                            start=(kc == 0), stop=(kc == 3),
                        )
                    if biases_zero:
                        if m % 2 == 0:
                            nc.vector.tensor_copy(embW0_sb[:, m, :, :], ew_ps[:])
                        else:
                            nc.scalar.copy(embW0_sb[:, m, :, :], ew_ps[:])
                    else:
                        nc.scalar.activation(embW0_sb[:, m, :, :], ew_ps[:],
                                             ACTF.Identity, bias=bias0_sb[:, m:m + 1])

                for b in range(BL):
                    for hc in range(4):
                        ep_ps = sup.tile([128, T], F32, tag="epps")
                        for ec in range(4):
                            nc.tensor.matmul(
                                ep_ps[:],
                                wattnT_sb[:, ec, hc, :],
                                encE_sb[:, ec, b, :],
                                start=(ec == 0), stop=(ec == 3),
                            )
                        if (b + hc) % 2 == 0:
                            nc.vector.tensor_copy(encP_sb[:, hc, b, :], ep_ps[:])
                        else:
                            nc.scalar.copy(encP_sb[:, hc, b, :], ep_ps[:])

            # ---- phase 1: recurrence ----
            with (
                tc.tile_pool(name="p1s", bufs=1) as p1s,
                tc.tile_pool(name="p1p", bufs=1, space="PSUM") as p1p,
            ):
                # persistent PSUM tiles; scores rotate on u parity for the
                # mask pre-accumulation trick
                sc_ps = [p1p.tile([128, T], F32, tag=f"sc{i}", name=f"sc{i}")
                         for i in range(2)]
                atT_ps = p1p.tile([128, 4, 128], BF16, tag="atT")
                ctx_ps = p1p.tile([128, E], F32, tag="ctx")
                ctT_ps = p1p.tile([128, 4, 128], BF16, tag="ctT")
                gates0_ps = p1p.tile([128, 16, BL], F32, tag="gates0")
                gates1_ps = p1p.tile([128, 16, BL], F32, tag="gates1")

                att_sb = p1s.tile([128, T], BF16, tag="att")
                ssum = p1s.tile([128, 1], F32, tag="ssum")
                rec = p1s.tile([128, 1], F32, tag="rec")
                atT_sb = p1s.tile([128, 4, BL], BF16, tag="atTsb")
                ctx_sb = p1s.tile([128, E], BF16, tag="ctxsb")
                g0_sb = p1s.tile([128, 8, BL], F32, tag="g0")
                t0_sb = p1s.tile([128, 8, BL], F32, tag="t0")
                ni0_sb = p1s.tile([128, 4, BL], F32, tag="ni0")
                a0_sb = p1s.tile([128, 4, BL], F32, tag="a0")
                np0_sb = p1s.tile([128, 4, BL], F32, tag="np0")
                n0_sb = p1s.tile([128, 4, BL], F32, tag="n0")
                d0_sb = p1s.tile([128, 4, BL], F32, tag="d0")
                g1_sb = p1s.tile([128, 8, BL], F32, tag="g1")
                t1_sb = p1s.tile([128, 8, BL], F32, tag="t1")
                a1_sb = p1s.tile([128, 4, BL], F32, tag="a1")
                np1_sb = p1s.tile([128, 4, BL], F32, tag="np1")
                n1_sb = p1s.tile([128, 4, BL], F32, tag="n1")
                d1_sb = p1s.tile([128, 4, BL], F32, tag="d1")
                ones_sb = p1s.tile([1, 1], BF16, tag="ones")
                ones128_sb = p1s.tile([1, 128], BF16, tag="ones128")
                zrow_sb = p1s.tile([1, T], BF16, tag="zrow")
                nc.gpsimd.memset(ones_sb[:], 1.0)
                nc.gpsimd.memset(ones128_sb[:], 1.0)
                nc.gpsimd.memset(zrow_sb[:], 0.0)

                # one-time init: write every partition row of the score/ctx
                # banks so never-again-written rows hold 0, not pre-kernel
                # garbage (exp/transpose would otherwise see inf/NaN there).
                for i in range(2):
                    nc.tensor.matmul(
                        sc_ps[i][:, :], ones128_sb[:], zrow_sb[:],
                        start=True, stop=False, skip_group_check=True,
                    )
                nc.tensor.matmul(
                    ctx_ps[:, :], ones128_sb[:], zrow_sb[:, 0:E],
                    start=True, stop=False, skip_group_check=True,
                )

                # mask pre-accumulation for u=0
                for b in range(BL):
                    nc.tensor.matmul(
                        sc_ps[0][32 * b:32 * b + 1, :],
                        ones_sb[:], mask_sb[:, b, :],
                        start=True, stop=False, skip_group_check=True,
                        tile_position=(0, 32 * b),
                    )

                def mm_nh(gps, wsb, rhs_fn, u):
                    for m in range(4):
                        for k in range(4):
                            nc.tensor.matmul(
                                gps[:, 12 + m, :], wsb[:, k, m, :], rhs_fn(k),
                                start=(k == 0), stop=(k == 3),
                            )

                def mm_rz_h(gps, wsb, rhs_fn, u):
                    for m in range(8):
                        for k in range(4):
                            nc.tensor.matmul(
                                gps[:, m, :], wsb[:, k, m, :], rhs_fn(k),
                                start=(k == 0), stop=False,
                            )

                def h0rhs_fn(u):
                    return (lambda k: zero_sb[:, k, :]) if u == 0 else \
                           (lambda k: h0b[:, k, :])

                def h1rhs_fn(u):
                    return (lambda k: zero_sb[:, k, :]) if u == 0 else \
                           (lambda k: yT_sb[:, k, u - 1, :])

                # u=0 h-dependent GRU0 contractions (zeros)
                mm_nh(gates0_ps, wn0h_sb, h0rhs_fn(0), 0)
                mm_rz_h(gates0_ps, wrz0_sb, h0rhs_fn(0), 0)

                for u in range(u_steps):
                    cur = sc_ps[u % 2]
                    nxt = sc_ps[(u + 1) % 2]

                    # scores[b, t] += sum_h q[h, b] * encP[b][h, t]
                    for b in range(BL):
                        for kc in range(4):
                            lhs = (embT_sb[:, kc, 0, b:b + 1] if u == 0
                                   else yT_sb[:, kc, u - 1, b:b + 1])
                            nc.tensor.matmul(
                                cur[32 * b:32 * b + 1, :],
                                lhs,
                                encP_sb[:, kc, b, :],
                                start=False, stop=(kc == 3),
                                skip_group_check=True,
                                tile_position=(0, 32 * b),
                            )

                    # GRU1 h1-dependent contractions fill the softmax gap
                    mm_nh(gates1_ps, wn1h_sb, h1rhs_fn(u), u)
                    mm_rz_h(gates1_ps, wrz1_sb, h1rhs_fn(u), u)

                    # softmax (no max-subtract; mask rows are -1e30)
                    nc.scalar.activation(att_sb[:], cur[:], ACTF.Exp,
                                         accum_out=ssum[:])
                    nc.vector.reciprocal(rec[:], ssum[:])

                    # attT: batch b sits in column 32b; keep those columns
                    for tc4 in range(4):
                        nc.tensor.transpose(
                            atT_ps[:, tc4, :],
                            att_sb[:, tc4 * 128:(tc4 + 1) * 128],
                            identb_sb[:],
                        )
                    nc.vector.tensor_copy(atT_sb[:], atT_ps[:, :, 0:128:32])

                    # ctx[b, e] += att[b, t] * encT[b][t, e]
                    for b in range(BL):
                        for tc4 in range(4):
                            nc.tensor.matmul(
                                ctx_ps[32 * b:32 * b + 1, :],
                                atT_sb[:, tc4, b:b + 1],
                                encT_sb[:, tc4, b, :],
                                start=(tc4 == 0), stop=(tc4 == 3),
                                skip_group_check=True,
                                tile_position=(0, 32 * b),
                            )

                    # mask pre-accumulation for u+1 fills the ctx-copy gap
                    if u + 1 < u_steps:
                        for b in range(BL):
                            nc.tensor.matmul(
                                nxt[32 * b:32 * b + 1, :],
                                ones_sb[:], mask_sb[:, b, :],
                                start=True, stop=False, skip_group_check=True,
                                tile_position=(0, 32 * b),
                            )

                    # ctx normalize-on-copy (scale = 1/sum per batch row),
                    # split across scalar+vector engines
                    nc.vector.tensor_scalar_mul(ctx_sb[:, 0:E // 2],
                                                ctx_ps[:, 0:E // 2], rec[:])
                    nc.scalar.activation(ctx_sb[:, E // 2:E],
                                         ctx_ps[:, E // 2:E], ACTF.Copy,
                                         scale=rec[:])
                    for ec in range(4):
                        nc.tensor.transpose(
                            ctT_ps[:, ec, :],
                            ctx_sb[:, ec * 128:(ec + 1) * 128],
                            identb_sb[:],
                        )
                    nc.vector.tensor_copy(yT_sb[:, 4:8, u, :],
                                          ctT_ps[:, :, 0:128:32])

                    # GRU0 ctx-dependent contractions
                    for m in range(8):
                        for k in range(4):
                            nc.tensor.matmul(
                                gates0_ps[:, m, :],
                                wrz0_sb[:, 4 + k, m, :],
                                yT_sb[:, 4 + k, u, :],
                                start=False, stop=(k == 3),
                            )
                    for m in range(4):
                        for k in range(4):
                            nc.tensor.matmul(
                                gates0_ps[:, 8 + m, :],
                                wn0i_sb[:, k, m, :],
                                yT_sb[:, 4 + k, u, :],
                                start=(k == 0), stop=(k == 3),
                            )

                    # ---- GRU0 gate math ([128, m, b] layout) ----
                    nc.vector.tensor_tensor(g0_sb[:], gates0_ps[:, 0:8, :],
                                            embW0_sb[:, 0:8, u, :], op=ALU.add)
                    nc.scalar.activation(t0_sb[:], g0_sb[:], ACTF.Tanh,
                                         scale=0.5)
                    nc.vector.tensor_tensor(ni0_sb[:], gates0_ps[:, 8:12, :],
                                            embW0_sb[:, 8:12, u, :], op=ALU.add)
                    if biases_zero:
                        nc.vector.scalar_tensor_tensor(
                            a0_sb[:], t0_sb[:, 0:4, :], 1.0,
                            gates0_ps[:, 12:16, :],
                            op0=ALU.add, op1=ALU.mult)
                    else:
                        nc.vector.tensor_tensor(a0_sb[:],
                                                gates0_ps[:, 12:16, :],
                                                bnh0_sb[:], op=ALU.add)
                        nc.vector.scalar_tensor_tensor(
                            a0_sb[:], t0_sb[:, 0:4, :], 1.0, a0_sb[:],
                            op0=ALU.add, op1=ALU.mult)
                    nc.vector.scalar_tensor_tensor(
                        np0_sb[:], a0_sb[:], 0.5, ni0_sb[:],
                        op0=ALU.mult, op1=ALU.add)
                    nc.scalar.activation(n0_sb[:], np0_sb[:], ACTF.Tanh)
                    nc.vector.tensor_tensor(d0_sb[:], h0b[:], n0_sb[:],
                                            op=ALU.subtract)
                    nc.vector.scalar_tensor_tensor(
                        d0_sb[:], t0_sb[:, 4:8, :], 1.0, d0_sb[:],
                        op0=ALU.add, op1=ALU.mult)
                    nc.vector.scalar_tensor_tensor(
                        h0b[:], d0_sb[:], 0.5, n0_sb[:],
                        op0=ALU.mult, op1=ALU.add)

                    # GRU1 h0n-dependent contractions
                    for m in range(8):
                        for k in range(4):
                            nc.tensor.matmul(
                                gates1_ps[:, m, :],
                                wrz1_sb[:, 4 + k, m, :],
                                h0b[:, k, :],
                                start=False, stop=(k == 3),
                            )
                    for m in range(4):
                        for k in range(4):
                            nc.tensor.matmul(
                                gates1_ps[:, 8 + m, :],
                                wn1i_sb[:, k, m, :],
                                h0b[:, k, :],
                                start=(k == 0), stop=(k == 3),
                            )

                    # next step's h0-dependent GRU0 contractions fill the
                    # GRU1 gate-math gap
                    if u + 1 < u_steps:
                        mm_nh(gates0_ps, wn0h_sb, h0rhs_fn(u + 1), u + 1)
                        mm_rz_h(gates0_ps, wrz0_sb, h0rhs_fn(u + 1), u + 1)

                    # ---- GRU1 gate math ----
                    if biases_zero:
                        nc.scalar.activation(t1_sb[:], gates1_ps[:, 0:8, :],
                                             ACTF.Tanh, scale=0.5)
                        nc.vector.scalar_tensor_tensor(
                            a1_sb[:], t1_sb[:, 0:4, :], 1.0,
                            gates1_ps[:, 12:16, :],
                            op0=ALU.add, op1=ALU.mult)
                        nc.vector.scalar_tensor_tensor(
                            np1_sb[:], a1_sb[:], 0.5, gates1_ps[:, 8:12, :],
                            op0=ALU.mult, op1=ALU.add)
                    else:
                        nc.vector.tensor_tensor(g1_sb[:], gates1_ps[:, 0:8, :],
                                                brz1_sb[:], op=ALU.add)
                        nc.scalar.activation(t1_sb[:], g1_sb[:], ACTF.Tanh,
                                             scale=0.5)
                        nc.vector.tensor_tensor(a1_sb[:],
                                                gates1_ps[:, 12:16, :],
                                                bnh1_sb[:], op=ALU.add)
                        nc.vector.scalar_tensor_tensor(
                            a1_sb[:], t1_sb[:, 0:4, :], 1.0, a1_sb[:],
                            op0=ALU.add, op1=ALU.mult)
                        nc.vector.tensor_tensor(np1_sb[:],
                                                gates1_ps[:, 8:12, :],
                                                bni1_sb[:], op=ALU.add)
                        nc.vector.scalar_tensor_tensor(
                            np1_sb[:], a1_sb[:], 0.5, np1_sb[:],
                            op0=ALU.mult, op1=ALU.add)
                    nc.scalar.activation(n1_sb[:], np1_sb[:], ACTF.Tanh)
                    d1_rhs = (zero_sb[:, :, :] if u == 0
                              else yT_sb[:, 0:4, u - 1, :])
                    nc.vector.tensor_tensor(d1_sb[:], d1_rhs, n1_sb[:],
                                            op=ALU.subtract)
                    nc.vector.scalar_tensor_tensor(
                        d1_sb[:], t1_sb[:, 4:8, :], 1.0, d1_sb[:],
                        op0=ALU.add, op1=ALU.mult)
                    nc.vector.scalar_tensor_tensor(
                        yT_sb[:, 0:4, u, :], d1_sb[:], 0.5, n1_sb[:],
                        op0=ALU.mult, op1=ALU.add)

            # ---- phase 2: full-vocab projection for the local batches ----
            with (
                tc.tile_pool(name="p2w", bufs=2) as p2w,
                tc.tile_pool(name="p2o", bufs=2) as p2o,
                tc.tile_pool(name="p2p", bufs=4, space="PSUM") as p2p,
            ):
                for s in range(NSUP):
                    if s < NPRE:
                        wt = wpre_sb[s]
                    else:
                        wt = p2w.tile([128, 8, 8, 128], BF16, tag="wt")
                        nc.sync.dma_start(wt[:], woutT_d.ap()[s])
                    ob = p2o.tile([128, 8, UB_L], BF16, tag="ob")
                    for vc in range(8):
                        ps = p2p.tile([128, UB_L], F32, tag="p2")
                        for kc in range(8):
                            nc.tensor.matmul(
                                ps[:],
                                wt[:, vc, kc, :],
                                yT_sb[:, kc, :, :],
                                start=(kc == 0), stop=(kc == 7),
                            )
                        if vc % 2 == 0:
                            nc.scalar.activation(
                                ob[:, vc, :], ps[:], ACTF.Identity,
                                bias=bout_sb[:, s * 8 + vc:s * 8 + vc + 1])
                        else:
                            nc.vector.tensor_scalar_add(
                                ob[:, vc, :], ps[:],
                                bout_sb[:, s * 8 + vc:s * 8 + vc + 1])
                    nc.sync.dma_start(out_d.ap()[s], ob[:])

    nc.finalize()
    return nc


_NC_CACHE = {}


def _get_nc(biases_zero=True):
    if biases_zero not in _NC_CACHE:
        _NC_CACHE[biases_zero] = build_nc(biases_zero=biases_zero)
    return _NC_CACHE[biases_zero]


def make_in_maps(inputs):
    f32 = np.float32
    bf = ml_dtypes.bfloat16
    enc = np.asarray(inputs["encoder_out"], f32)
    lens = np.asarray(inputs["encoder_lens"]).astype(np.int64)
    dec = np.asarray(inputs["decoder_in"]).astype(np.int64)
    emb_table = np.asarray(inputs["emb_table"], f32)
    W_attn = np.asarray(inputs["W_attn"], f32)
    W_ih0 = np.asarray(inputs["W_ih0"], f32)
    W_hh0 = np.asarray(inputs["W_hh0"], f32)
    b_ih0 = np.asarray(inputs["b_ih0"], f32)
    b_hh0 = np.asarray(inputs["b_hh0"], f32)
    W_ih1 = np.asarray(inputs["W_ih1"], f32)
    W_hh1 = np.asarray(inputs["W_hh1"], f32)
    b_ih1 = np.asarray(inputs["b_ih1"], f32)
    b_hh1 = np.asarray(inputs["b_hh1"], f32)
    W_out = np.asarray(inputs["W_out"], f32)
    b_out = np.asarray(inputs["b_out"], f32)

    embedded = emb_table[dec]                       # [B, U, H]
    mask = np.where(
        np.arange(T)[None, :] >= lens[:, None],
        f32(-1e30), f32(0.0))                       # [B, T]

    def chunkT(w):
        # [K, M] weight -> lhsT chunks [128, kc, mc, 128] (bf16)
        K, M = w.shape
        return np.ascontiguousarray(
            w.reshape(K // 128, 128, M // 128, 128).transpose(1, 0, 2, 3)
        ).astype(bf)

    # per-step GRU lhsT chunk tables; k-order: h-part first, then ctx/x-part
    wrz0 = np.concatenate([W_hh0[0:1024].T, W_ih0[0:1024, 512:1024].T], 0)
    wrz0 = chunkT(wrz0)                             # [128, 8, 8, 128]
    wn0i = chunkT(W_ih0[1024:1536, 512:1024].T)
    wn0h = chunkT(W_hh0[1024:1536].T)
    wrz1 = np.concatenate([W_hh1[0:1024].T, W_ih1[0:1024].T], 0)
    wrz1 = chunkT(wrz1)
    wn1i = chunkT(W_ih1[1024:1536].T)
    wn1h = chunkT(W_hh1[1024:1536].T)
    wemb0 = chunkT(W_ih0[:, 0:512].T)               # [128, 4, 12, 128]
    wattnT = chunkT(W_attn.T)                       # [128, 4ec, 4hc, 128]

    Wp = np.zeros((VP, 1024), f32)
    Wp[:V] = W_out
    woutT = np.ascontiguousarray(
        Wp.reshape(NSUP, 8, 128, 8, 128).transpose(0, 4, 1, 3, 2)
    ).astype(bf)                                    # [32, 128k, 8vc, 8kc, 128v]
    bp = np.zeros((VP,), f32)
    bp[:V] = b_out
    bout_t = np.ascontiguousarray(bp.reshape(NSUP * 8, 128).T)

    # biases
    bias0 = np.zeros((128, 12), f32)                # embW0 bias (rz: ih+hh, n_i: ih)
    brz = (b_ih0[:1024] + b_hh0[:1024]).reshape(8, 128).T
    bias0[:, 0:8] = brz
    bias0[:, 8:12] = b_ih0[1024:1536].reshape(4, 128).T
    bcast = lambda v: np.ascontiguousarray(np.broadcast_to(
        v.reshape(v.shape[0] // 128, 128).T[:, :, None], (128, v.shape[0] // 128, BL)))
    brz1 = bcast(b_ih1[:1024] + b_hh1[:1024])
    bnh0 = bcast(b_hh0[1024:1536])
    bni1 = bcast(b_ih1[1024:1536])
    bnh1 = bcast(b_hh1[1024:1536])

    identb = np.eye(128, dtype=f32).astype(bf)

    in_maps = []
    for c in range(NCORES):
        bs = slice(BL * c, BL * (c + 1))
        encl = enc[bs]                              # [BL, T, E]
        encE = np.ascontiguousarray(
            encl.transpose(2, 0, 1).reshape(4, 128, BL, T).transpose(1, 0, 2, 3)
        ).astype(bf)                                # [128, 4ec, BL, T]
        encTt = np.ascontiguousarray(
            encl.transpose(1, 0, 2).reshape(4, 128, BL, E).transpose(1, 0, 2, 3)
        ).astype(bf)                                # [128, 4tc, BL, E]
        embT = np.ascontiguousarray(
            embedded[bs].transpose(2, 1, 0).reshape(4, 128, U, BL).transpose(1, 0, 2, 3)
        ).astype(bf)                                # [128, 4hc, U, BL]
        in_maps.append({
            "encE": encE,
            "encT": encTt,
            "embT": embT,
            "mask": np.ascontiguousarray(mask[bs][None, :, :]).astype(bf),
            "wattnT": wattnT,
            "wemb0": wemb0,
            "wrz0": wrz0, "wn0i": wn0i, "wn0h": wn0h,
            "wrz1": wrz1, "wn1i": wn1i, "wn1h": wn1h,
            "woutT": woutT,
            "bout": bout_t,
            "identb": identb,
            "bias0": bias0,
            "brz1": brz1, "bnh0": bnh0, "bni1": bni1, "bnh1": bnh1,
        })
    return in_maps


def assemble_output(results):
    logits = np.zeros((B, U, V), np.float32)
    for c in range(NCORES):
        o = np.asarray(results[c]["out"], np.float32)  # [32, 128, 8, U, BL]
        o = o.transpose(4, 3, 0, 2, 1).reshape(BL, U, VP)
        logits[BL * c:BL * (c + 1)] = o[:, :, :V]
    return logits


def kernel(**inputs):
    bz = all(
        float(np.abs(np.asarray(inputs[k])).max()) == 0.0
        for k in ("b_ih0", "b_hh0", "b_ih1", "b_hh1")
    )
    nc = _get_nc(biases_zero=bz)
    in_maps = make_in_maps(inputs)
    res = run_bass_kernel_spmd(nc, in_maps, core_ids=list(range(NCORES)))
    return assemble_output(res.results)


if __name__ == "__main__":
    nc = build_nc()
    print("built OK")


# revision 17
# speedup vs baseline: 2.8959x; 1.0114x over previous
"""AttentionDecoder Trainium2 kernel (8 NeuronCores).

Strategy (v2):
  - Batch-shard everything: core c owns batches [4c, 4c+4). No collectives.
  - enc_proj = W_attn-projected encoder is precomputed once per core, and the
    embedding contribution to GRU-layer-0 gates (+ its biases) is precomputed
    for all 64 steps in one GEMM, so the per-step recurrence only contracts
    ctx/h terms.
  - The recurrence keeps every activation in [feature-on-partitions, batch]
    layout. GRU matmuls run with the weight chunk as the 128x128 stationary
    operand (bf16, fast-weight-load) and the 4-wide activations moving, so
    gates land as [gate_dim, batch] and all gate math is short-free-dim
    DVE/ACT ops. sigmoid(x) = 0.5*tanh(x/2)+0.5 keeps the scalar engine on
    one activation table (exp+tanh) forever.
  - Attention scores for the 4 batches accumulate into one PSUM bank at
    partitions {0,32,64,96}; the length mask joins the accumulation as a
    1-row matmul issued a step early; softmax skips max-subtraction and the
    1/sum normalization is folded into the ctx PSUM->SBUF copy as a
    per-partition scale.
  - Y^T = [h1; ctx] accumulates in SBUF in bf16. Phase 2 computes the full
    vocab for the local 4 batches, streaming W_out^T (bf16) from HBM in
    2 MB super-tiles that double-buffer against the GEMM.
"""

import numpy as np
import ml_dtypes

import concourse.bass as bass
import concourse.bacc as bacc_mod
import concourse.mybir as mybir
from concourse import tile
from concourse.bass_utils import run_bass_kernel_spmd

B, T, U = 32, 512, 64
V, H, E = 32000, 512, 512
NCORES = 8
BL = B // NCORES          # local batches per core
NSUP = 32                 # phase-2 vocab super-tiles (8 x 128 vocab each)
VP = NSUP * 8 * 128       # padded vocab (32768)
UB_L = U * BL             # 256 local (u, b) columns

F32 = mybir.dt.float32
F32R = mybir.dt.float32r
BF16 = mybir.dt.bfloat16
AX = mybir.AxisListType
ALU = mybir.AluOpType
ACTF = mybir.ActivationFunctionType


def build_nc(u_steps=U, biases_zero=True):
    nc = bacc_mod.Bacc()

    encE_d = nc.declare_dram_parameter("encE", [128, 4, BL, T], BF16, isOutput=False)
    encT_d = nc.declare_dram_parameter("encT", [128, 4, BL, E], BF16, isOutput=False)
    embT_d = nc.declare_dram_parameter("embT", [128, 4, U, BL], BF16, isOutput=False)
    mask_d = nc.declare_dram_parameter("mask", [1, BL, T], BF16, isOutput=False)
    wattnT_d = nc.declare_dram_parameter("wattnT", [128, 4, 4, 128], BF16, isOutput=False)
    wemb0_d = nc.declare_dram_parameter("wemb0", [128, 4, 12, 128], BF16, isOutput=False)
    wrz0_d = nc.declare_dram_parameter("wrz0", [128, 8, 8, 128], BF16, isOutput=False)
    wn0i_d = nc.declare_dram_parameter("wn0i", [128, 4, 4, 128], BF16, isOutput=False)
    wn0h_d = nc.declare_dram_parameter("wn0h", [128, 4, 4, 128], BF16, isOutput=False)
    wrz1_d = nc.declare_dram_parameter("wrz1", [128, 8, 8, 128], BF16, isOutput=False)
    wn1i_d = nc.declare_dram_parameter("wn1i", [128, 4, 4, 128], BF16, isOutput=False)
    wn1h_d = nc.declare_dram_parameter("wn1h", [128, 4, 4, 128], BF16, isOutput=False)
    woutT_d = nc.declare_dram_parameter("woutT", [NSUP, 128, 8, 8, 128], BF16, isOutput=False)
    bout_d = nc.declare_dram_parameter("bout", [128, NSUP * 8], F32, isOutput=False)
    identb_d = nc.declare_dram_parameter("identb", [128, 128], BF16, isOutput=False)
    bias0_d = nc.declare_dram_parameter("bias0", [128, 12], F32, isOutput=False)
    brz1_d = nc.declare_dram_parameter("brz1", [128, 8, BL], F32, isOutput=False)
    bnh0_d = nc.declare_dram_parameter("bnh0", [128, 4, BL], F32, isOutput=False)
    bni1_d = nc.declare_dram_parameter("bni1", [128, 4, BL], F32, isOutput=False)
    bnh1_d = nc.declare_dram_parameter("bnh1", [128, 4, BL], F32, isOutput=False)
    out_d = nc.declare_dram_parameter("out", [NSUP, 128, 8, U, BL], BF16, isOutput=True)

    with tile.TileContext(nc) as tc:
        with tc.tile_pool(name="res", bufs=1) as res:
            # ---- resident SBUF ----
            encT_sb = res.tile([128, 4, BL, E], BF16, tag="encT")
            nc.sync.dma_start(encT_sb[:], encT_d.ap())
            embT_sb = res.tile([128, 4, U, BL], BF16, tag="embT")
            nc.sync.dma_start(embT_sb[:], embT_d.ap())
            mask_sb = res.tile([1, BL, T], BF16, tag="mask")
            nc.sync.dma_start(mask_sb[:], mask_d.ap())
            wrz0_sb = res.tile([128, 8, 8, 128], BF16, tag="wrz0")
            nc.sync.dma_start(wrz0_sb[:], wrz0_d.ap())
            wn0i_sb = res.tile([128, 4, 4, 128], BF16, tag="wn0i")
            nc.sync.dma_start(wn0i_sb[:], wn0i_d.ap())
            wn0h_sb = res.tile([128, 4, 4, 128], BF16, tag="wn0h")
            nc.sync.dma_start(wn0h_sb[:], wn0h_d.ap())
            wrz1_sb = res.tile([128, 8, 8, 128], BF16, tag="wrz1")
            nc.sync.dma_start(wrz1_sb[:], wrz1_d.ap())
            wn1i_sb = res.tile([128, 4, 4, 128], BF16, tag="wn1i")
            nc.sync.dma_start(wn1i_sb[:], wn1i_d.ap())
            wn1h_sb = res.tile([128, 4, 4, 128], BF16, tag="wn1h")
            nc.sync.dma_start(wn1h_sb[:], wn1h_d.ap())
            identb_sb = res.tile([128, 128], BF16, tag="identb")
            nc.sync.dma_start(identb_sb[:], identb_d.ap())
            bout_sb = res.tile([128, NSUP * 8], F32, tag="bout")
            nc.sync.dma_start(bout_sb[:], bout_d.ap())
            if not biases_zero:
                bias0_sb = res.tile([128, 12], F32, tag="bias0")
                nc.sync.dma_start(bias0_sb[:], bias0_d.ap())
                brz1_sb = res.tile([128, 8, BL], F32, tag="brz1")
                nc.sync.dma_start(brz1_sb[:], brz1_d.ap())
                bnh0_sb = res.tile([128, 4, BL], F32, tag="bnh0")
                nc.sync.dma_start(bnh0_sb[:], bnh0_d.ap())
                bni1_sb = res.tile([128, 4, BL], F32, tag="bni1")
                nc.sync.dma_start(bni1_sb[:], bni1_d.ap())
                bnh1_sb = res.tile([128, 4, BL], F32, tag="bnh1")
                nc.sync.dma_start(bnh1_sb[:], bnh1_d.ap())

            NPRE = 2
            wpre_sb = [res.tile([128, 8, 8, 128], BF16, tag=f"wpre{i}",
                                name=f"wpre{i}")
                       for i in range(NPRE)]
            for i in range(NPRE):
                nc.sync.dma_start(wpre_sb[i][:], woutT_d.ap()[i])

            encP_sb = res.tile([128, 4, BL, T], BF16, tag="encP")
            embW0_sb = res.tile([128, 12, U, BL], F32, tag="embW0")
            yT_sb = res.tile([128, 8, U, BL], BF16, tag="yT")

            # persistent recurrence state (h kept in bf16)
            h0b = res.tile([128, 4, BL], BF16, tag="h0b")
            zero_sb = res.tile([128, 4, BL], BF16, tag="zero")
            nc.gpsimd.memset(h0b[:], 0.0)
            nc.gpsimd.memset(zero_sb[:], 0.0)

            # ---- setup: encP = W_attn^T-projected enc; embW0 = Wih0_emb @ emb ----
            with (
                tc.tile_pool(name="su", bufs=1) as su,
                tc.tile_pool(name="sup", bufs=2, space="PSUM") as sup,
            ):
                encE_sb = su.tile([128, 4, BL, T], BF16, tag="encE")
                nc.sync.dma_start(encE_sb[:], encE_d.ap())
                wattnT_sb = su.tile([128, 4, 4, 128], BF16, tag="wattnT")
                nc.sync.dma_start(wattnT_sb[:], wattnT_d.ap())
                wemb0_sb = su.tile([128, 4, 12, 128], BF16, tag="wemb0")
                nc.sync.dma_start(wemb0_sb[:], wemb0_d.ap())

                for m in range(12):
                    ew_ps = sup.tile([128, U * BL], F32, tag="ewps")
                    for kc in range(4):
                        nc.tensor.matmul(
                            ew_ps[:],
                            wemb0_sb[:, kc, m, :],
                            embT_sb[:, kc, :, :],
                            start=(kc == 0), stop=(kc == 3),
                        )
                    if biases_zero:
                        if m % 2 == 0:
                            nc.vector.tensor_copy(embW0_sb[:, m, :, :], ew_ps[:])
                        else:
                            nc.scalar.copy(embW0_sb[:, m, :, :], ew_ps[:])
                    else:
                        nc.scalar.activation(embW0_sb[:, m, :, :], ew_ps[:],
                                             ACTF.Identity, bias=bias0_sb[:, m:m + 1])

                for b in range(BL):
                    for hc in range(4):
                        ep_ps = sup.tile([128, T], F32, tag="epps")
                        for ec in range(4):
                            nc.tensor.matmul(
                                ep_ps[:],
                                wattnT_sb[:, ec, hc, :],
                                encE_sb[:, ec, b, :],
                                start=(ec == 0), stop=(ec == 3),
                            )
                        if (b + hc) % 2 == 0:
                            nc.vector.tensor_copy(encP_sb[:, hc, b, :], ep_ps[:])
                        else:
                            nc.scalar.copy(encP_sb[:, hc, b, :], ep_ps[:])

            # ---- phase 1: recurrence ----
            with (
                tc.tile_pool(name="p1s", bufs=1) as p1s,
                tc.tile_pool(name="p1p", bufs=1, space="PSUM") as p1p,
            ):
                # persistent PSUM tiles; scores rotate on u parity for the
                # mask pre-accumulation trick
                sc_ps = [p1p.tile([128, T], F32, tag=f"sc{i}", name=f"sc{i}")
                         for i in range(2)]
                tT_ps = p1p.tile([128, 8, 128], BF16, tag="tT")
                ctx_ps = p1p.tile([128, E], F32, tag="ctx")
                rz0_ps = p1p.tile([128, 8, BL], F32, tag="rz0")
                nih0_ps = p1p.tile([128, 8, BL], F32, tag="nih0")
                rz1_ps = p1p.tile([128, 8, BL], F32, tag="rz1")
                nih1_ps = p1p.tile([128, 8, BL], F32, tag="nih1")

                att_sb = p1s.tile([128, T], BF16, tag="att")
                ssum = p1s.tile([128, 1], F32, tag="ssum")
                rec = p1s.tile([128, 1], F32, tag="rec")
                atT_sb = p1s.tile([128, 4, BL], BF16, tag="atTsb")
                ctx_sb = p1s.tile([128, E], BF16, tag="ctxsb")
                g0_sb = p1s.tile([128, 8, BL], F32, tag="g0")
                t0_sb = p1s.tile([128, 8, BL], F32, tag="t0")
                ni0_sb = p1s.tile([128, 4, BL], F32, tag="ni0")
                a0_sb = p1s.tile([128, 4, BL], F32, tag="a0")
                np0_sb = p1s.tile([128, 4, BL], F32, tag="np0")
                n0_sb = p1s.tile([128, 4, BL], F32, tag="n0")
                d0_sb = p1s.tile([128, 4, BL], F32, tag="d0")
                g1_sb = p1s.tile([128, 8, BL], F32, tag="g1")
                t1_sb = p1s.tile([128, 8, BL], F32, tag="t1")
                a1_sb = p1s.tile([128, 4, BL], F32, tag="a1")
                np1_sb = p1s.tile([128, 4, BL], F32, tag="np1")
                n1_sb = p1s.tile([128, 4, BL], F32, tag="n1")
                d1_sb = p1s.tile([128, 4, BL], F32, tag="d1")
                ones_sb = p1s.tile([1, 1], BF16, tag="ones")
                ones128_sb = p1s.tile([1, 128], BF16, tag="ones128")
                zrow_sb = p1s.tile([1, T], BF16, tag="zrow")
                nc.gpsimd.memset(ones_sb[:], 1.0)
                nc.gpsimd.memset(ones128_sb[:], 1.0)
                nc.gpsimd.memset(zrow_sb[:], 0.0)

                # one-time init: write every partition row of the score/ctx
                # banks so never-again-written rows hold 0, not pre-kernel
                # garbage (exp/transpose would otherwise see inf/NaN there).
                for i in range(2):
                    nc.tensor.matmul(
                        sc_ps[i][:, :], ones128_sb[:], zrow_sb[:],
                        start=True, stop=False, skip_group_check=True,
                    )
                nc.tensor.matmul(
                    ctx_ps[:, :], ones128_sb[:], zrow_sb[:, 0:E],
                    start=True, stop=False, skip_group_check=True,
                )

                # mask pre-accumulation for u=0
                for b in range(BL):
                    nc.tensor.matmul(
                        sc_ps[0][32 * b:32 * b + 1, :],
                        ones_sb[:], mask_sb[:, b, :],
                        start=True, stop=False, skip_group_check=True,
                        tile_position=(0, 32 * b),
                    )

                def mm_nh(gps, wsb, rhs_fn, u):
                    for m in range(4):
                        for k in range(4):
                            nc.tensor.matmul(
                                gps[:, 4 + m, :], wsb[:, k, m, :], rhs_fn(k),
                                start=(k == 0), stop=(k == 3),
                            )

                def mm_rz_h(gps, wsb, rhs_fn, u):
                    for m in range(8):
                        for k in range(4):
                            nc.tensor.matmul(
                                gps[:, m, :], wsb[:, k, m, :], rhs_fn(k),
                                start=(k == 0), stop=False,
                            )

                def h0rhs_fn(u):
                    return (lambda k: zero_sb[:, k, :]) if u == 0 else \
                           (lambda k: h0b[:, k, :])

                def h1rhs_fn(u):
                    return (lambda k: zero_sb[:, k, :]) if u == 0 else \
                           (lambda k: yT_sb[:, k, u - 1, :])

                # u=0 h-dependent GRU0 contractions (zeros)
                mm_nh(nih0_ps, wn0h_sb, h0rhs_fn(0), 0)
                mm_rz_h(rz0_ps, wrz0_sb, h0rhs_fn(0), 0)

                for u in range(u_steps):
                    cur = sc_ps[u % 2]
                    nxt = sc_ps[(u + 1) % 2]

                    # scores[b, t] += sum_h q[h, b] * encP[b][h, t]
                    for b in range(BL):
                        for kc in range(4):
                            lhs = (embT_sb[:, kc, 0, b:b + 1] if u == 0
                                   else yT_sb[:, kc, u - 1, b:b + 1])
                            nc.tensor.matmul(
                                cur[32 * b:32 * b + 1, :],
                                lhs,
                                encP_sb[:, kc, b, :],
                                start=False, stop=(kc == 3),
                                skip_group_check=True,
                                tile_position=(0, 32 * b),
                            )

                    # GRU1 h1-dependent contractions fill the softmax gap
                    mm_nh(nih1_ps, wn1h_sb, h1rhs_fn(u), u)
                    mm_rz_h(rz1_ps, wrz1_sb, h1rhs_fn(u), u)

                    # softmax (no max-subtract; mask rows are -1e30)
                    nc.scalar.activation(att_sb[:], cur[:], ACTF.Exp,
                                         accum_out=ssum[:])
                    nc.vector.reciprocal(rec[:], ssum[:])

                    # attT: batch b sits in column 32b; keep those columns
                    for tc4 in range(4):
                        nc.tensor.transpose(
                            tT_ps[:, tc4, :],
                            att_sb[:, tc4 * 128:(tc4 + 1) * 128],
                            identb_sb[:],
                        )
                    nc.vector.tensor_copy(atT_sb[:], tT_ps[:, 0:4, 0:128:32])

                    # ctx[b, e] += att[b, t] * encT[b][t, e]
                    for b in range(BL):
                        for tc4 in range(4):
                            nc.tensor.matmul(
                                ctx_ps[32 * b:32 * b + 1, :],
                                atT_sb[:, tc4, b:b + 1],
                                encT_sb[:, tc4, b, :],
                                start=(tc4 == 0), stop=(tc4 == 3),
                                skip_group_check=True,
                                tile_position=(0, 32 * b),
                            )

                    # ctx normalize-on-copy (scale = 1/sum per batch row),
                    # split across scalar+vector engines
                    nc.vector.tensor_scalar_mul(ctx_sb[:, 0:E // 2],
                                                ctx_ps[:, 0:E // 2], rec[:])
                    nc.scalar.activation(ctx_sb[:, E // 2:E],
                                         ctx_ps[:, E // 2:E], ACTF.Copy,
                                         scale=rec[:])
                    # ctxT transposes then GRU0 ctx contractions
                    for ec in range(4):
                        nc.tensor.transpose(
                            tT_ps[:, 4 + ec, :],
                            ctx_sb[:, ec * 128:(ec + 1) * 128],
                            identb_sb[:],
                        )
                    nc.vector.tensor_copy(yT_sb[:, 4:8, u, :],
                                          tT_ps[:, 4:8, 0:128:32])
                    for m in range(8):
                        for k in range(4):
                            nc.tensor.matmul(
                                rz0_ps[:, m, :],
                                wrz0_sb[:, 4 + k, m, :],
                                yT_sb[:, 4 + k, u, :],
                                start=False, stop=(k == 3),
                            )
                    for m in range(4):
                        for k in range(4):
                            nc.tensor.matmul(
                                nih0_ps[:, m, :],
                                wn0i_sb[:, k, m, :],
                                yT_sb[:, 4 + k, u, :],
                                start=(k == 0), stop=(k == 3),
                            )

                    # mask pre-accumulation for u+1 fills the gate0 gap
                    if u + 1 < u_steps:
                        for b in range(BL):
                            nc.tensor.matmul(
                                nxt[32 * b:32 * b + 1, :],
                                ones_sb[:], mask_sb[:, b, :],
                                start=True, stop=False, skip_group_check=True,
                                tile_position=(0, 32 * b),
                            )

                    # ---- GRU0 gate math ([128, m, b] layout) ----
                    nc.vector.tensor_tensor(g0_sb[:], rz0_ps[:],
                                            embW0_sb[:, 0:8, u, :], op=ALU.add)
                    nc.scalar.activation(t0_sb[:], g0_sb[:], ACTF.Tanh,
                                         scale=0.5)
                    nc.vector.tensor_tensor(ni0_sb[:], nih0_ps[:, 0:4, :],
                                            embW0_sb[:, 8:12, u, :], op=ALU.add)
                    if biases_zero:
                        nc.vector.scalar_tensor_tensor(
                            a0_sb[:], t0_sb[:, 0:4, :], 1.0,
                            nih0_ps[:, 4:8, :],
                            op0=ALU.add, op1=ALU.mult)
                    else:
                        nc.vector.tensor_tensor(a0_sb[:],
                                                nih0_ps[:, 4:8, :],
                                                bnh0_sb[:], op=ALU.add)
                        nc.vector.scalar_tensor_tensor(
                            a0_sb[:], t0_sb[:, 0:4, :], 1.0, a0_sb[:],
                            op0=ALU.add, op1=ALU.mult)
                    nc.vector.scalar_tensor_tensor(
                        np0_sb[:], a0_sb[:], 0.5, ni0_sb[:],
                        op0=ALU.mult, op1=ALU.add)
                    nc.scalar.activation(n0_sb[:], np0_sb[:], ACTF.Tanh)
                    nc.vector.tensor_tensor(d0_sb[:], h0b[:], n0_sb[:],
                                            op=ALU.subtract)
                    nc.vector.scalar_tensor_tensor(
                        d0_sb[:], t0_sb[:, 4:8, :], 1.0, d0_sb[:],
                        op0=ALU.add, op1=ALU.mult)
                    nc.vector.scalar_tensor_tensor(
                        h0b[:], d0_sb[:], 0.5, n0_sb[:],
                        op0=ALU.mult, op1=ALU.add)

                    # GRU1 h0n-dependent contractions
                    for m in range(8):
                        for k in range(4):
                            nc.tensor.matmul(
                                rz1_ps[:, m, :],
                                wrz1_sb[:, 4 + k, m, :],
                                h0b[:, k, :],
                                start=False, stop=(k == 3),
                            )
                    for m in range(4):
                        for k in range(4):
                            nc.tensor.matmul(
                                nih1_ps[:, m, :],
                                wn1i_sb[:, k, m, :],
                                h0b[:, k, :],
                                start=(k == 0), stop=(k == 3),
                            )

                    # next step's h0-dependent GRU0 contractions fill the
                    # GRU1 gate-math gap
                    if u + 1 < u_steps:
                        mm_nh(nih0_ps, wn0h_sb, h0rhs_fn(u + 1), u + 1)
                        mm_rz_h(rz0_ps, wrz0_sb, h0rhs_fn(u + 1), u + 1)

                    # ---- GRU1 gate math ----
                    if biases_zero:
                        nc.scalar.activation(t1_sb[:], rz1_ps[:],
                                             ACTF.Tanh, scale=0.5)
                        nc.vector.scalar_tensor_tensor(
                            a1_sb[:], t1_sb[:, 0:4, :], 1.0,
                            nih1_ps[:, 4:8, :],
                            op0=ALU.add, op1=ALU.mult)
                        nc.vector.scalar_tensor_tensor(
                            np1_sb[:], a1_sb[:], 0.5, nih1_ps[:, 0:4, :],
                            op0=ALU.mult, op1=ALU.add)
                    else:
                        nc.vector.tensor_tensor(g1_sb[:], rz1_ps[:],
                                                brz1_sb[:], op=ALU.add)
                        nc.scalar.activation(t1_sb[:], g1_sb[:], ACTF.Tanh,
                                             scale=0.5)
                        nc.vector.tensor_tensor(a1_sb[:],
                                                nih1_ps[:, 4:8, :],
                                                bnh1_sb[:], op=ALU.add)
                        nc.vector.scalar_tensor_tensor(
                            a1_sb[:], t1_sb[:, 0:4, :], 1.0, a1_sb[:],
                            op0=ALU.add, op1=ALU.mult)
                        nc.vector.tensor_tensor(np1_sb[:],
                                                nih1_ps[:, 0:4, :],
                                                bni1_sb[:], op=ALU.add)
                        nc.vector.scalar_tensor_tensor(
                            np1_sb[:], a1_sb[:], 0.5, np1_sb[:],
                            op0=ALU.mult, op1=ALU.add)
                    nc.scalar.activation(n1_sb[:], np1_sb[:], ACTF.Tanh)
                    d1_rhs = (zero_sb[:, :, :] if u == 0
                              else yT_sb[:, 0:4, u - 1, :])
                    nc.vector.tensor_tensor(d1_sb[:], d1_rhs, n1_sb[:],
                                            op=ALU.subtract)
                    nc.vector.scalar_tensor_tensor(
                        d1_sb[:], t1_sb[:, 4:8, :], 1.0, d1_sb[:],
                        op0=ALU.add, op1=ALU.mult)
                    nc.vector.scalar_tensor_tensor(
                        yT_sb[:, 0:4, u, :], d1_sb[:], 0.5, n1_sb[:],
                        op0=ALU.mult, op1=ALU.add)

            # ---- phase 2: full-vocab projection for the local batches ----
            with (
                tc.tile_pool(name="p2w", bufs=2) as p2w,
                tc.tile_pool(name="p2o", bufs=2) as p2o,
                tc.tile_pool(name="p2p", bufs=4, space="PSUM") as p2p,
            ):
                for s in range(NSUP):
                    if s < NPRE:
                        wt = wpre_sb[s]
                    else:
                        wt = p2w.tile([128, 8, 8, 128], BF16, tag="wt")
                        nc.sync.dma_start(wt[:], woutT_d.ap()[s])
                    ob = p2o.tile([128, 8, UB_L], BF16, tag="ob")
                    for vc in range(8):
                        ps = p2p.tile([128, UB_L], F32, tag="p2")
                        for kc in range(8):
                            nc.tensor.matmul(
                                ps[:],
                                wt[:, vc, kc, :],
                                yT_sb[:, kc, :, :],
                                start=(kc == 0), stop=(kc == 7),
                            )
                        if vc % 2 == 0:
                            nc.scalar.activation(
                                ob[:, vc, :], ps[:], ACTF.Identity,
                                bias=bout_sb[:, s * 8 + vc:s * 8 + vc + 1])
                        else:
                            nc.vector.tensor_scalar_add(
                                ob[:, vc, :], ps[:],
                                bout_sb[:, s * 8 + vc:s * 8 + vc + 1])
                    nc.sync.dma_start(out_d.ap()[s], ob[:])

    nc.finalize()
    return nc


_NC_CACHE = {}


def _get_nc(biases_zero=True):
    if biases_zero not in _NC_CACHE:
        _NC_CACHE[biases_zero] = build_nc(biases_zero=biases_zero)
    return _NC_CACHE[biases_zero]


def make_in_maps(inputs):
    f32 = np.float32
    bf = ml_dtypes.bfloat16
    enc = np.asarray(inputs["encoder_out"], f32)
    lens = np.asarray(inputs["encoder_lens"]).astype(np.int64)
    dec = np.asarray(inputs["decoder_in"]).astype(np.int64)
    emb_table = np.asarray(inputs["emb_table"], f32)
    W_attn = np.asarray(inputs["W_attn"], f32)
    W_ih0 = np.asarray(inputs["W_ih0"], f32)
    W_hh0 = np.asarray(inputs["W_hh0"], f32)
    b_ih0 = np.asarray(inputs["b_ih0"], f32)
    b_hh0 = np.asarray(inputs["b_hh0"], f32)
    W_ih1 = np.asarray(inputs["W_ih1"], f32)
    W_hh1 = np.asarray(inputs["W_hh1"], f32)
    b_ih1 = np.asarray(inputs["b_ih1"], f32)
    b_hh1 = np.asarray(inputs["b_hh1"], f32)
    W_out = np.asarray(inputs["W_out"], f32)
    b_out = np.asarray(inputs["b_out"], f32)

    embedded = emb_table[dec]                       # [B, U, H]
    mask = np.where(
        np.arange(T)[None, :] >= lens[:, None],
        f32(-1e30), f32(0.0))                       # [B, T]

    def chunkT(w):
        # [K, M] weight -> lhsT chunks [128, kc, mc, 128] (bf16)
        K, M = w.shape
        return np.ascontiguousarray(
            w.reshape(K // 128, 128, M // 128, 128).transpose(1, 0, 2, 3)
        ).astype(bf)

    # per-step GRU lhsT chunk tables; k-order: h-part first, then ctx/x-part
    wrz0 = np.concatenate([W_hh0[0:1024].T, W_ih0[0:1024, 512:1024].T], 0)
    wrz0 = chunkT(wrz0)                             # [128, 8, 8, 128]
    wn0i = chunkT(W_ih0[1024:1536, 512:1024].T)
    wn0h = chunkT(W_hh0[1024:1536].T)
    wrz1 = np.concatenate([W_hh1[0:1024].T, W_ih1[0:1024].T], 0)
    wrz1 = chunkT(wrz1)
    wn1i = chunkT(W_ih1[1024:1536].T)
    wn1h = chunkT(W_hh1[1024:1536].T)
    wemb0 = chunkT(W_ih0[:, 0:512].T)               # [128, 4, 12, 128]
    wattnT = chunkT(W_attn.T)                       # [128, 4ec, 4hc, 128]

    Wp = np.zeros((VP, 1024), f32)
    Wp[:V] = W_out
    woutT = np.ascontiguousarray(
        Wp.reshape(NSUP, 8, 128, 8, 128).transpose(0, 4, 1, 3, 2)
    ).astype(bf)                                    # [32, 128k, 8vc, 8kc, 128v]
    bp = np.zeros((VP,), f32)
    bp[:V] = b_out
    bout_t = np.ascontiguousarray(bp.reshape(NSUP * 8, 128).T)

    # biases
    bias0 = np.zeros((128, 12), f32)                # embW0 bias (rz: ih+hh, n_i: ih)
    brz = (b_ih0[:1024] + b_hh0[:1024]).reshape(8, 128).T
    bias0[:, 0:8] = brz
    bias0[:, 8:12] = b_ih0[1024:1536].reshape(4, 128).T
    bcast = lambda v: np.ascontiguousarray(np.broadcast_to(
        v.reshape(v.shape[0] // 128, 128).T[:, :, None], (128, v.shape[0] // 128, BL)))
    brz1 = bcast(b_ih1[:1024] + b_hh1[:1024])
    bnh0 = bcast(b_hh0[1024:1536])
    bni1 = bcast(b_ih1[1024:1536])
    bnh1 = bcast(b_hh1[1024:1536])

    identb = np.eye(128, dtype=f32).astype(bf)

    in_maps = []
    for c in range(NCORES):
        bs = slice(BL * c, BL * (c + 1))
        encl = enc[bs]                              # [BL, T, E]
        encE = np.ascontiguousarray(
            encl.transpose(2, 0, 1).reshape(4, 128, BL, T).transpose(1, 0, 2, 3)
        ).astype(bf)                                # [128, 4ec, BL, T]
        encTt = np.ascontiguousarray(
            encl.transpose(1, 0, 2).reshape(4, 128, BL, E).transpose(1, 0, 2, 3)
        ).astype(bf)                                # [128, 4tc, BL, E]
        embT = np.ascontiguousarray(
            embedded[bs].transpose(2, 1, 0).reshape(4, 128, U, BL).transpose(1, 0, 2, 3)
        ).astype(bf)                                # [128, 4hc, U, BL]
        in_maps.append({
            "encE": encE,
            "encT": encTt,
            "embT": embT,
            "mask": np.ascontiguousarray(mask[bs][None, :, :]).astype(bf),
            "wattnT": wattnT,
            "wemb0": wemb0,
            "wrz0": wrz0, "wn0i": wn0i, "wn0h": wn0h,
            "wrz1": wrz1, "wn1i": wn1i, "wn1h": wn1h,
            "woutT": woutT,
            "bout": bout_t,
            "identb": identb,
            "bias0": bias0,
            "brz1": brz1, "bnh0": bnh0, "bni1": bni1, "bnh1": bnh1,
        })
    return in_maps


def assemble_output(results):
    logits = np.zeros((B, U, V), np.float32)
    for c in range(NCORES):
        o = np.asarray(results[c]["out"], np.float32)  # [32, 128, 8, U, BL]
        o = o.transpose(4, 3, 0, 2, 1).reshape(BL, U, VP)
        logits[BL * c:BL * (c + 1)] = o[:, :, :V]
    return logits


def kernel(**inputs):
    bz = all(
        float(np.abs(np.asarray(inputs[k])).max()) == 0.0
        for k in ("b_ih0", "b_hh0", "b_ih1", "b_hh1")
    )
    nc = _get_nc(biases_zero=bz)
    in_maps = make_in_maps(inputs)
    res = run_bass_kernel_spmd(nc, in_maps, core_ids=list(range(NCORES)))
    return assemble_output(res.results)


if __name__ == "__main__":
    nc = build_nc()
    print("built OK")


# revision 18
# speedup vs baseline: 2.9403x; 1.0153x over previous
"""AttentionDecoder Trainium2 kernel (8 NeuronCores).

Strategy (v2):
  - Batch-shard everything: core c owns batches [4c, 4c+4). No collectives.
  - enc_proj = W_attn-projected encoder is precomputed once per core, and the
    embedding contribution to GRU-layer-0 gates (+ its biases) is precomputed
    for all 64 steps in one GEMM, so the per-step recurrence only contracts
    ctx/h terms.
  - The recurrence keeps every activation in [feature-on-partitions, batch]
    layout. GRU matmuls run with the weight chunk as the 128x128 stationary
    operand (bf16, fast-weight-load) and the 4-wide activations moving, so
    gates land as [gate_dim, batch] and all gate math is short-free-dim
    DVE/ACT ops. sigmoid(x) = 0.5*tanh(x/2)+0.5 keeps the scalar engine on
    one activation table (exp+tanh) forever.
  - Attention scores for the 4 batches accumulate into one PSUM bank at
    partitions {0,32,64,96}; the length mask joins the accumulation as a
    1-row matmul issued a step early; softmax skips max-subtraction and the
    1/sum normalization is folded into the ctx PSUM->SBUF copy as a
    per-partition scale.
  - Y^T = [h1; ctx] accumulates in SBUF in bf16. Phase 2 computes the full
    vocab for the local 4 batches, streaming W_out^T (bf16) from HBM in
    2 MB super-tiles that double-buffer against the GEMM.
"""

import numpy as np
import ml_dtypes

import concourse.bass as bass
import concourse.bacc as bacc_mod
import concourse.mybir as mybir
from concourse import tile
from concourse.bass_utils import run_bass_kernel_spmd

B, T, U = 32, 512, 64
V, H, E = 32000, 512, 512
NCORES = 8
BL = B // NCORES          # local batches per core
NSUP = 32                 # phase-2 vocab super-tiles (8 x 128 vocab each)
VP = NSUP * 8 * 128       # padded vocab (32768)
UB_L = U * BL             # 256 local (u, b) columns

F32 = mybir.dt.float32
F32R = mybir.dt.float32r
BF16 = mybir.dt.bfloat16
AX = mybir.AxisListType
ALU = mybir.AluOpType
ACTF = mybir.ActivationFunctionType


def build_nc(u_steps=U, biases_zero=True):
    nc = bacc_mod.Bacc()

    encE_d = nc.declare_dram_parameter("encE", [128, 4, BL, T], BF16, isOutput=False)
    encT_d = nc.declare_dram_parameter("encT", [128, 4, BL, E], BF16, isOutput=False)
    embT_d = nc.declare_dram_parameter("embT", [128, 4, U, BL], BF16, isOutput=False)
    mask_d = nc.declare_dram_parameter("mask", [1, BL, T], BF16, isOutput=False)
    wattnT_d = nc.declare_dram_parameter("wattnT", [128, 4, 4, 128], BF16, isOutput=False)
    wemb0_d = nc.declare_dram_parameter("wemb0", [128, 4, 12, 128], BF16, isOutput=False)
    wrz0_d = nc.declare_dram_parameter("wrz0", [128, 8, 8, 128], BF16, isOutput=False)
    wn0i_d = nc.declare_dram_parameter("wn0i", [128, 4, 4, 128], BF16, isOutput=False)
    wn0h_d = nc.declare_dram_parameter("wn0h", [128, 4, 4, 128], BF16, isOutput=False)
    wrz1_d = nc.declare_dram_parameter("wrz1", [128, 8, 8, 128], BF16, isOutput=False)
    wn1i_d = nc.declare_dram_parameter("wn1i", [128, 4, 4, 128], BF16, isOutput=False)
    wn1h_d = nc.declare_dram_parameter("wn1h", [128, 4, 4, 128], BF16, isOutput=False)
    woutT_d = nc.declare_dram_parameter("woutT", [NSUP, 128, 8, 8, 128], BF16, isOutput=False)
    bout_d = nc.declare_dram_parameter("bout", [128, NSUP * 8], F32, isOutput=False)
    identb_d = nc.declare_dram_parameter("identb", [128, 128], BF16, isOutput=False)
    bias0_d = nc.declare_dram_parameter("bias0", [128, 12], F32, isOutput=False)
    brz1_d = nc.declare_dram_parameter("brz1", [128, 8, BL], F32, isOutput=False)
    bnh0_d = nc.declare_dram_parameter("bnh0", [128, 4, BL], F32, isOutput=False)
    bni1_d = nc.declare_dram_parameter("bni1", [128, 4, BL], F32, isOutput=False)
    bnh1_d = nc.declare_dram_parameter("bnh1", [128, 4, BL], F32, isOutput=False)
    out_d = nc.declare_dram_parameter("out", [NSUP, 128, 8, U, BL], BF16, isOutput=True)

    with tile.TileContext(nc) as tc:
        with tc.tile_pool(name="res", bufs=1) as res:
            # ---- resident SBUF ----
            encT_sb = res.tile([128, 4, BL, E], BF16, tag="encT")
            nc.sync.dma_start(encT_sb[:], encT_d.ap())
            embT_sb = res.tile([128, 4, U, BL], BF16, tag="embT")
            nc.sync.dma_start(embT_sb[:], embT_d.ap())
            mask_sb = res.tile([1, BL, T], BF16, tag="mask")
            nc.sync.dma_start(mask_sb[:], mask_d.ap())
            wrz0_sb = res.tile([128, 8, 8, 128], BF16, tag="wrz0")
            nc.sync.dma_start(wrz0_sb[:], wrz0_d.ap())
            wn0i_sb = res.tile([128, 4, 4, 128], BF16, tag="wn0i")
            nc.sync.dma_start(wn0i_sb[:], wn0i_d.ap())
            wn0h_sb = res.tile([128, 4, 4, 128], BF16, tag="wn0h")
            nc.sync.dma_start(wn0h_sb[:], wn0h_d.ap())
            wrz1_sb = res.tile([128, 8, 8, 128], BF16, tag="wrz1")
            nc.sync.dma_start(wrz1_sb[:], wrz1_d.ap())
            wn1i_sb = res.tile([128, 4, 4, 128], BF16, tag="wn1i")
            nc.sync.dma_start(wn1i_sb[:], wn1i_d.ap())
            wn1h_sb = res.tile([128, 4, 4, 128], BF16, tag="wn1h")
            nc.sync.dma_start(wn1h_sb[:], wn1h_d.ap())
            identb_sb = res.tile([128, 128], BF16, tag="identb")
            nc.sync.dma_start(identb_sb[:], identb_d.ap())
            bout_sb = res.tile([128, NSUP * 8], F32, tag="bout")
            nc.sync.dma_start(bout_sb[:], bout_d.ap())
            if not biases_zero:
                bias0_sb = res.tile([128, 12], F32, tag="bias0")
                nc.sync.dma_start(bias0_sb[:], bias0_d.ap())
                brz1_sb = res.tile([128, 8, BL], F32, tag="brz1")
                nc.sync.dma_start(brz1_sb[:], brz1_d.ap())
                bnh0_sb = res.tile([128, 4, BL], F32, tag="bnh0")
                nc.sync.dma_start(bnh0_sb[:], bnh0_d.ap())
                bni1_sb = res.tile([128, 4, BL], F32, tag="bni1")
                nc.sync.dma_start(bni1_sb[:], bni1_d.ap())
                bnh1_sb = res.tile([128, 4, BL], F32, tag="bnh1")
                nc.sync.dma_start(bnh1_sb[:], bnh1_d.ap())

            NPRE = 2
            wpre_sb = [res.tile([128, 8, 8, 128], BF16, tag=f"wpre{i}",
                                name=f"wpre{i}")
                       for i in range(NPRE)]
            for i in range(NPRE):
                nc.sync.dma_start(wpre_sb[i][:], woutT_d.ap()[i])

            encP_sb = res.tile([128, 4, BL, T], BF16, tag="encP")
            embW0_sb = res.tile([128, 12, U, BL], F32, tag="embW0")
            yT_sb = res.tile([128, 8, U, BL], BF16, tag="yT")

            # persistent recurrence state (h kept in bf16)
            h0b = res.tile([128, 4, BL], BF16, tag="h0b")
            zero_sb = res.tile([128, 4, BL], BF16, tag="zero")
            nc.gpsimd.memset(h0b[:], 0.0)
            nc.gpsimd.memset(zero_sb[:], 0.0)

            # ---- setup: encP = W_attn^T-projected enc; embW0 = Wih0_emb @ emb ----
            with (
                tc.tile_pool(name="su", bufs=1) as su,
                tc.tile_pool(name="sup", bufs=2, space="PSUM") as sup,
            ):
                encE_sb = su.tile([128, 4, BL, T], BF16, tag="encE")
                nc.sync.dma_start(encE_sb[:], encE_d.ap())
                wattnT_sb = su.tile([128, 4, 4, 128], BF16, tag="wattnT")
                nc.sync.dma_start(wattnT_sb[:], wattnT_d.ap())
                wemb0_sb = su.tile([128, 4, 12, 128], BF16, tag="wemb0")
                nc.sync.dma_start(wemb0_sb[:], wemb0_d.ap())

                for m in range(12):
                    ew_ps = sup.tile([128, U * BL], F32, tag="ewps")
                    for kc in range(4):
                        nc.tensor.matmul(
                            ew_ps[:],
                            wemb0_sb[:, kc, m, :],
                            embT_sb[:, kc, :, :],
                            start=(kc == 0), stop=(kc == 3),
                        )
                    if biases_zero:
                        if m % 2 == 0:
                            nc.vector.tensor_copy(embW0_sb[:, m, :, :], ew_ps[:])
                        else:
                            nc.scalar.copy(embW0_sb[:, m, :, :], ew_ps[:])
                    else:
                        nc.scalar.activation(embW0_sb[:, m, :, :], ew_ps[:],
                                             ACTF.Identity, bias=bias0_sb[:, m:m + 1])

                for b in range(BL):
                    for hc in range(4):
                        ep_ps = sup.tile([128, T], F32, tag="epps")
                        for ec in range(4):
                            nc.tensor.matmul(
                                ep_ps[:],
                                wattnT_sb[:, ec, hc, :],
                                encE_sb[:, ec, b, :],
                                start=(ec == 0), stop=(ec == 3),
                            )
                        if (b + hc) % 2 == 0:
                            nc.vector.tensor_copy(encP_sb[:, hc, b, :], ep_ps[:])
                        else:
                            nc.scalar.copy(encP_sb[:, hc, b, :], ep_ps[:])

            # ---- phase 1: recurrence ----
            with (
                tc.tile_pool(name="p1s", bufs=1) as p1s,
                tc.tile_pool(name="p1p", bufs=1, space="PSUM") as p1p,
            ):
                # persistent PSUM tiles; scores rotate on u parity for the
                # mask pre-accumulation trick
                sc_ps = [p1p.tile([128, T], F32, tag=f"sc{i}", name=f"sc{i}")
                         for i in range(2)]
                tT_ps = p1p.tile([128, 8, 128], BF16, tag="tT")
                ctx_ps = p1p.tile([128, E], F32, tag="ctx")
                rz0_ps = p1p.tile([128, 8, BL], F32, tag="rz0")
                nih0_ps = p1p.tile([128, 8, BL], F32, tag="nih0")
                rz1_ps = p1p.tile([128, 8, BL], F32, tag="rz1")
                nih1_ps = p1p.tile([128, 8, BL], F32, tag="nih1")

                att_sb = p1s.tile([128, T], BF16, tag="att")
                ssum = p1s.tile([128, 1], F32, tag="ssum")
                rec = p1s.tile([128, 1], F32, tag="rec")
                atT_sb = p1s.tile([128, 4, BL], BF16, tag="atTsb")
                ctx_sb = p1s.tile([128, E], BF16, tag="ctxsb")
                g0_sb = p1s.tile([128, 8, BL], F32, tag="g0")
                t0_sb = p1s.tile([128, 8, BL], F32, tag="t0")
                ni0_sb = p1s.tile([128, 4, BL], F32, tag="ni0")
                a0_sb = p1s.tile([128, 4, BL], F32, tag="a0")
                np0_sb = p1s.tile([128, 4, BL], F32, tag="np0")
                n0_sb = p1s.tile([128, 4, BL], F32, tag="n0")
                d0_sb = p1s.tile([128, 4, BL], F32, tag="d0")
                g1_sb = p1s.tile([128, 8, BL], F32, tag="g1")
                t1_sb = p1s.tile([128, 8, BL], F32, tag="t1")
                a1_sb = p1s.tile([128, 4, BL], F32, tag="a1")
                np1_sb = p1s.tile([128, 4, BL], F32, tag="np1")
                n1_sb = p1s.tile([128, 4, BL], F32, tag="n1")
                d1_sb = p1s.tile([128, 4, BL], F32, tag="d1")
                ones_sb = p1s.tile([1, 1], BF16, tag="ones")
                ones128_sb = p1s.tile([1, 128], BF16, tag="ones128")
                zrow_sb = p1s.tile([1, T], BF16, tag="zrow")
                nc.gpsimd.memset(ones_sb[:], 1.0)
                nc.gpsimd.memset(ones128_sb[:], 1.0)
                nc.gpsimd.memset(zrow_sb[:], 0.0)

                # one-time init: write every partition row of the score/ctx
                # banks so never-again-written rows hold 0, not pre-kernel
                # garbage (exp/transpose would otherwise see inf/NaN there).
                for i in range(2):
                    nc.tensor.matmul(
                        sc_ps[i][:, :], ones128_sb[:], zrow_sb[:],
                        start=True, stop=False, skip_group_check=True,
                    )
                nc.tensor.matmul(
                    ctx_ps[:, :], ones128_sb[:], zrow_sb[:, 0:E],
                    start=True, stop=False, skip_group_check=True,
                )

                # mask pre-accumulation for u=0
                for b in range(BL):
                    nc.tensor.matmul(
                        sc_ps[0][32 * b:32 * b + 1, :],
                        ones_sb[:], mask_sb[:, b, :],
                        start=True, stop=False, skip_group_check=True,
                        tile_position=(0, 32 * b),
                    )

                def mm_nh(gps, wsb, rhs_fn, u):
                    for m in range(4):
                        for k in range(4):
                            nc.tensor.matmul(
                                gps[:, 4 + m, :], wsb[:, k, m, :], rhs_fn(k),
                                start=(k == 0), stop=(k == 3),
                            )

                def mm_rz_h(gps, wsb, rhs_fn, u, ms=range(8)):
                    for m in ms:
                        for k in range(4):
                            nc.tensor.matmul(
                                gps[:, m, :], wsb[:, k, m, :], rhs_fn(k),
                                start=(k == 0), stop=False,
                            )

                def h0rhs_fn(u):
                    return (lambda k: zero_sb[:, k, :]) if u == 0 else \
                           (lambda k: h0b[:, k, :])

                def h1rhs_fn(u):
                    return (lambda k: zero_sb[:, k, :]) if u == 0 else \
                           (lambda k: yT_sb[:, k, u - 1, :])

                # u=0 h-dependent GRU0 contractions (zeros)
                mm_nh(nih0_ps, wn0h_sb, h0rhs_fn(0), 0)
                mm_rz_h(rz0_ps, wrz0_sb, h0rhs_fn(0), 0)

                for u in range(u_steps):
                    cur = sc_ps[u % 2]
                    nxt = sc_ps[(u + 1) % 2]

                    # scores[b, t] += sum_h q[h, b] * encP[b][h, t]
                    for b in range(BL):
                        for kc in range(4):
                            lhs = (embT_sb[:, kc, 0, b:b + 1] if u == 0
                                   else yT_sb[:, kc, u - 1, b:b + 1])
                            nc.tensor.matmul(
                                cur[32 * b:32 * b + 1, :],
                                lhs,
                                encP_sb[:, kc, b, :],
                                start=False, stop=(kc == 3),
                                skip_group_check=True,
                                tile_position=(0, 32 * b),
                            )

                    # GRU1 h1-dependent contractions fill the softmax gap
                    mm_nh(nih1_ps, wn1h_sb, h1rhs_fn(u), u)
                    mm_rz_h(rz1_ps, wrz1_sb, h1rhs_fn(u), u, ms=range(4))

                    # softmax (no max-subtract; mask rows are -1e30)
                    nc.scalar.activation(att_sb[:], cur[:], ACTF.Exp,
                                         accum_out=ssum[:])
                    nc.vector.reciprocal(rec[:], ssum[:])

                    # attT: batch b sits in column 32b; keep those columns
                    for tc4 in range(4):
                        nc.tensor.transpose(
                            tT_ps[:, tc4, :],
                            att_sb[:, tc4 * 128:(tc4 + 1) * 128],
                            identb_sb[:],
                        )
                    nc.vector.tensor_copy(atT_sb[:], tT_ps[:, 0:4, 0:128:32])

                    # ctx[b, e] += att[b, t] * encT[b][t, e]
                    for b in range(BL):
                        for tc4 in range(4):
                            nc.tensor.matmul(
                                ctx_ps[32 * b:32 * b + 1, :],
                                atT_sb[:, tc4, b:b + 1],
                                encT_sb[:, tc4, b, :],
                                start=(tc4 == 0), stop=(tc4 == 3),
                                skip_group_check=True,
                                tile_position=(0, 32 * b),
                            )

                    # ctx normalize-on-copy (scale = 1/sum per batch row),
                    # split across scalar+vector engines
                    nc.vector.tensor_scalar_mul(ctx_sb[:, 0:E // 2],
                                                ctx_ps[:, 0:E // 2], rec[:])
                    nc.scalar.activation(ctx_sb[:, E // 2:E],
                                         ctx_ps[:, E // 2:E], ACTF.Copy,
                                         scale=rec[:])
                    # ctxT transposes then GRU0 ctx contractions
                    for ec in range(4):
                        nc.tensor.transpose(
                            tT_ps[:, 4 + ec, :],
                            ctx_sb[:, ec * 128:(ec + 1) * 128],
                            identb_sb[:],
                        )
                    nc.vector.tensor_copy(yT_sb[:, 4:8, u, :],
                                          tT_ps[:, 4:8, 0:128:32])
                    for m in range(8):
                        for k in range(4):
                            nc.tensor.matmul(
                                rz0_ps[:, m, :],
                                wrz0_sb[:, 4 + k, m, :],
                                yT_sb[:, 4 + k, u, :],
                                start=False, stop=(k == 3),
                            )
                    for m in range(4):
                        for k in range(4):
                            nc.tensor.matmul(
                                nih0_ps[:, m, :],
                                wn0i_sb[:, k, m, :],
                                yT_sb[:, 4 + k, u, :],
                                start=(k == 0), stop=(k == 3),
                            )

                    # mask pre-accumulation for u+1 fills the gate0 gap
                    if u + 1 < u_steps:
                        for b in range(BL):
                            nc.tensor.matmul(
                                nxt[32 * b:32 * b + 1, :],
                                ones_sb[:], mask_sb[:, b, :],
                                start=True, stop=False, skip_group_check=True,
                                tile_position=(0, 32 * b),
                            )

                    # ---- GRU0 gate math ([128, m, b] layout) ----
                    nc.vector.tensor_tensor(g0_sb[:], rz0_ps[:],
                                            embW0_sb[:, 0:8, u, :], op=ALU.add)
                    nc.scalar.activation(t0_sb[:], g0_sb[:], ACTF.Tanh,
                                         scale=0.5)
                    nc.vector.tensor_tensor(ni0_sb[:], nih0_ps[:, 0:4, :],
                                            embW0_sb[:, 8:12, u, :], op=ALU.add)
                    if biases_zero:
                        nc.vector.scalar_tensor_tensor(
                            a0_sb[:], t0_sb[:, 0:4, :], 1.0,
                            nih0_ps[:, 4:8, :],
                            op0=ALU.add, op1=ALU.mult)
                    else:
                        nc.vector.tensor_tensor(a0_sb[:],
                                                nih0_ps[:, 4:8, :],
                                                bnh0_sb[:], op=ALU.add)
                        nc.vector.scalar_tensor_tensor(
                            a0_sb[:], t0_sb[:, 0:4, :], 1.0, a0_sb[:],
                            op0=ALU.add, op1=ALU.mult)
                    nc.vector.scalar_tensor_tensor(
                        np0_sb[:], a0_sb[:], 0.5, ni0_sb[:],
                        op0=ALU.mult, op1=ALU.add)
                    nc.scalar.activation(n0_sb[:], np0_sb[:], ACTF.Tanh)
                    nc.vector.tensor_tensor(d0_sb[:], h0b[:], n0_sb[:],
                                            op=ALU.subtract)
                    nc.vector.scalar_tensor_tensor(
                        d0_sb[:], t0_sb[:, 4:8, :], 1.0, d0_sb[:],
                        op0=ALU.add, op1=ALU.mult)
                    nc.vector.scalar_tensor_tensor(
                        h0b[:], d0_sb[:], 0.5, n0_sb[:],
                        op0=ALU.mult, op1=ALU.add)

                    # GRU1 h0n-dependent contractions
                    for m in range(8):
                        for k in range(4):
                            nc.tensor.matmul(
                                rz1_ps[:, m, :],
                                wrz1_sb[:, 4 + k, m, :],
                                h0b[:, k, :],
                                start=False, stop=(k == 3),
                            )
                    for m in range(4):
                        for k in range(4):
                            nc.tensor.matmul(
                                nih1_ps[:, m, :],
                                wn1i_sb[:, k, m, :],
                                h0b[:, k, :],
                                start=(k == 0), stop=(k == 3),
                            )

                    # next step's h0-dependent GRU0 contractions fill the
                    # GRU1 gate-math gap
                    if u + 1 < u_steps:
                        mm_nh(nih0_ps, wn0h_sb, h0rhs_fn(u + 1), u + 1)
                        mm_rz_h(rz0_ps, wrz0_sb, h0rhs_fn(u + 1), u + 1)

                    # ---- GRU1 gate math ----
                    if biases_zero:
                        nc.scalar.activation(t1_sb[:], rz1_ps[:],
                                             ACTF.Tanh, scale=0.5)
                        nc.vector.scalar_tensor_tensor(
                            a1_sb[:], t1_sb[:, 0:4, :], 1.0,
                            nih1_ps[:, 4:8, :],
                            op0=ALU.add, op1=ALU.mult)
                        nc.vector.scalar_tensor_tensor(
                            np1_sb[:], a1_sb[:], 0.5, nih1_ps[:, 0:4, :],
                            op0=ALU.mult, op1=ALU.add)
                    else:
                        nc.vector.tensor_tensor(g1_sb[:], rz1_ps[:],
                                                brz1_sb[:], op=ALU.add)
                        nc.scalar.activation(t1_sb[:], g1_sb[:], ACTF.Tanh,
                                             scale=0.5)
                        nc.vector.tensor_tensor(a1_sb[:],
                                                nih1_ps[:, 4:8, :],
                                                bnh1_sb[:], op=ALU.add)
                        nc.vector.scalar_tensor_tensor(
                            a1_sb[:], t1_sb[:, 0:4, :], 1.0, a1_sb[:],
                            op0=ALU.add, op1=ALU.mult)
                        nc.vector.tensor_tensor(np1_sb[:],
                                                nih1_ps[:, 0:4, :],
                                                bni1_sb[:], op=ALU.add)
                        nc.vector.scalar_tensor_tensor(
                            np1_sb[:], a1_sb[:], 0.5, np1_sb[:],
                            op0=ALU.mult, op1=ALU.add)
                    nc.scalar.activation(n1_sb[:], np1_sb[:], ACTF.Tanh)
                    d1_rhs = (zero_sb[:, :, :] if u == 0
                              else yT_sb[:, 0:4, u - 1, :])
                    nc.vector.tensor_tensor(d1_sb[:], d1_rhs, n1_sb[:],
                                            op=ALU.subtract)
                    nc.vector.scalar_tensor_tensor(
                        d1_sb[:], t1_sb[:, 4:8, :], 1.0, d1_sb[:],
                        op0=ALU.add, op1=ALU.mult)
                    nc.vector.scalar_tensor_tensor(
                        yT_sb[:, 0:4, u, :], d1_sb[:], 0.5, n1_sb[:],
                        op0=ALU.mult, op1=ALU.add)

            # ---- phase 2: full-vocab projection for the local batches ----
            with (
                tc.tile_pool(name="p2w", bufs=2) as p2w,
                tc.tile_pool(name="p2o", bufs=2) as p2o,
                tc.tile_pool(name="p2p", bufs=4, space="PSUM") as p2p,
            ):
                for s in range(NSUP):
                    if s < NPRE:
                        wt = wpre_sb[s]
                    else:
                        wt = p2w.tile([128, 8, 8, 128], BF16, tag="wt")
                        nc.sync.dma_start(wt[:], woutT_d.ap()[s])
                    ob = p2o.tile([128, 8, UB_L], BF16, tag="ob")
                    for vc in range(8):
                        ps = p2p.tile([128, UB_L], F32, tag="p2")
                        for kc in range(8):
                            nc.tensor.matmul(
                                ps[:],
                                wt[:, vc, kc, :],
                                yT_sb[:, kc, :, :],
                                start=(kc == 0), stop=(kc == 7),
                            )
                        if vc % 2 == 0:
                            nc.scalar.activation(
                                ob[:, vc, :], ps[:], ACTF.Identity,
                                bias=bout_sb[:, s * 8 + vc:s * 8 + vc + 1])
                        else:
                            nc.vector.tensor_scalar_add(
                                ob[:, vc, :], ps[:],
                                bout_sb[:, s * 8 + vc:s * 8 + vc + 1])
                    nc.sync.dma_start(out_d.ap()[s], ob[:])

    nc.finalize()
    return nc


_NC_CACHE = {}


def _get_nc(biases_zero=True):
    if biases_zero not in _NC_CACHE:
        _NC_CACHE[biases_zero] = build_nc(biases_zero=biases_zero)
    return _NC_CACHE[biases_zero]


def make_in_maps(inputs):
    f32 = np.float32
    bf = ml_dtypes.bfloat16
    enc = np.asarray(inputs["encoder_out"], f32)
    lens = np.asarray(inputs["encoder_lens"]).astype(np.int64)
    dec = np.asarray(inputs["decoder_in"]).astype(np.int64)
    emb_table = np.asarray(inputs["emb_table"], f32)
    W_attn = np.asarray(inputs["W_attn"], f32)
    W_ih0 = np.asarray(inputs["W_ih0"], f32)
    W_hh0 = np.asarray(inputs["W_hh0"], f32)
    b_ih0 = np.asarray(inputs["b_ih0"], f32)
    b_hh0 = np.asarray(inputs["b_hh0"], f32)
    W_ih1 = np.asarray(inputs["W_ih1"], f32)
    W_hh1 = np.asarray(inputs["W_hh1"], f32)
    b_ih1 = np.asarray(inputs["b_ih1"], f32)
    b_hh1 = np.asarray(inputs["b_hh1"], f32)
    W_out = np.asarray(inputs["W_out"], f32)
    b_out = np.asarray(inputs["b_out"], f32)

    embedded = emb_table[dec]                       # [B, U, H]
    mask = np.where(
        np.arange(T)[None, :] >= lens[:, None],
        f32(-1e30), f32(0.0))                       # [B, T]

    def chunkT(w):
        # [K, M] weight -> lhsT chunks [128, kc, mc, 128] (bf16)
        K, M = w.shape
        return np.ascontiguousarray(
            w.reshape(K // 128, 128, M // 128, 128).transpose(1, 0, 2, 3)
        ).astype(bf)

    # per-step GRU lhsT chunk tables; k-order: h-part first, then ctx/x-part
    wrz0 = np.concatenate([W_hh0[0:1024].T, W_ih0[0:1024, 512:1024].T], 0)
    wrz0 = chunkT(wrz0)                             # [128, 8, 8, 128]
    wn0i = chunkT(W_ih0[1024:1536, 512:1024].T)
    wn0h = chunkT(W_hh0[1024:1536].T)
    wrz1 = np.concatenate([W_hh1[0:1024].T, W_ih1[0:1024].T], 0)
    wrz1 = chunkT(wrz1)
    wn1i = chunkT(W_ih1[1024:1536].T)
    wn1h = chunkT(W_hh1[1024:1536].T)
    wemb0 = chunkT(W_ih0[:, 0:512].T)               # [128, 4, 12, 128]
    wattnT = chunkT(W_attn.T)                       # [128, 4ec, 4hc, 128]

    Wp = np.zeros((VP, 1024), f32)
    Wp[:V] = W_out
    woutT = np.ascontiguousarray(
        Wp.reshape(NSUP, 8, 128, 8, 128).transpose(0, 4, 1, 3, 2)
    ).astype(bf)                                    # [32, 128k, 8vc, 8kc, 128v]
    bp = np.zeros((VP,), f32)
    bp[:V] = b_out
    bout_t = np.ascontiguousarray(bp.reshape(NSUP * 8, 128).T)

    # biases
    bias0 = np.zeros((128, 12), f32)                # embW0 bias (rz: ih+hh, n_i: ih)
    brz = (b_ih0[:1024] + b_hh0[:1024]).reshape(8, 128).T
    bias0[:, 0:8] = brz
    bias0[:, 8:12] = b_ih0[1024:1536].reshape(4, 128).T
    bcast = lambda v: np.ascontiguousarray(np.broadcast_to(
        v.reshape(v.shape[0] // 128, 128).T[:, :, None], (128, v.shape[0] // 128, BL)))
    brz1 = bcast(b_ih1[:1024] + b_hh1[:1024])
    bnh0 = bcast(b_hh0[1024:1536])
    bni1 = bcast(b_ih1[1024:1536])
    bnh1 = bcast(b_hh1[1024:1536])

    identb = np.eye(128, dtype=f32).astype(bf)

    in_maps = []
    for c in range(NCORES):
        bs = slice(BL * c, BL * (c + 1))
        encl = enc[bs]                              # [BL, T, E]
        encE = np.ascontiguousarray(
            encl.transpose(2, 0, 1).reshape(4, 128, BL, T).transpose(1, 0, 2, 3)
        ).astype(bf)                                # [128, 4ec, BL, T]
        encTt = np.ascontiguousarray(
            encl.transpose(1, 0, 2).reshape(4, 128, BL, E).transpose(1, 0, 2, 3)
        ).astype(bf)                                # [128, 4tc, BL, E]
        embT = np.ascontiguousarray(
            embedded[bs].transpose(2, 1, 0).reshape(4, 128, U, BL).transpose(1, 0, 2, 3)
        ).astype(bf)                                # [128, 4hc, U, BL]
        in_maps.append({
            "encE": encE,
            "encT": encTt,
            "embT": embT,
            "mask": np.ascontiguousarray(mask[bs][None, :, :]).astype(bf),
            "wattnT": wattnT,
            "wemb0": wemb0,
            "wrz0": wrz0, "wn0i": wn0i, "wn0h": wn0h,
            "wrz1": wrz1, "wn1i": wn1i, "wn1h": wn1h,
            "woutT": woutT,
            "bout": bout_t,
            "identb": identb,
            "bias0": bias0,
            "brz1": brz1, "bnh0": bnh0, "bni1": bni1, "bnh1": bnh1,
        })
    return in_maps


def assemble_output(results):
    logits = np.zeros((B, U, V), np.float32)
    for c in range(NCORES):
        o = np.asarray(results[c]["out"], np.float32)  # [32, 128, 8, U, BL]
        o = o.transpose(4, 3, 0, 2, 1).reshape(BL, U, VP)
        logits[BL * c:BL * (c + 1)] = o[:, :, :V]
    return logits


def kernel(**inputs):
    bz = all(
        float(np.abs(np.asarray(inputs[k])).max()) == 0.0
        for k in ("b_ih0", "b_hh0", "b_ih1", "b_hh1")
    )
    nc = _get_nc(biases_zero=bz)
    in_maps = make_in_maps(inputs)
    res = run_bass_kernel_spmd(nc, in_maps, core_ids=list(range(NCORES)))
    return assemble_output(res.results)


if __name__ == "__main__":
    nc = build_nc()
    print("built OK")
